# revision 23
# baseline (speedup 1.0000x reference)
"""EngagementPredictor TRN2 kernel: 3-branch MHA + masked mean-pool + MLP.

Sharding: pure data-parallel - B=8 batch elements, one per NeuronCore;
weights replicated; no collectives. Each core computes its [2]-logit row.

v3 design (over the bf16 baseline):
  * Valid-token packing as before (Sp keys / Sq queries, zero-padded).
  * fp8(e4m3) DoubleRow matmuls for all Q/K/V projections: weights are
    host-quantized at x4096, x at x32, contraction pairs two 128-row
    k-tiles per instruction -> half the PE stream cycles.
  * Scores in [query-part, key-free] orientation (lhsT=Q). The exp
    activation produces the softmax denominator for free via accum_out
    (free-axis sum); padded keys have x=0 -> score 0 -> exp=1, their
    count is subtracted from the denominator and their V rows are
    exactly zero, so no mask tensor is needed at all.
  * Pooling: pooled[d] = sum_k V[k,d] * g[k], g[k] = sum_q exp[q,k] *
    (poolw[q]/den[q]); g and pooled are N=1 matmuls (no ctx
    materialization, no broadcast, no big DVE reductions).
  * tmp/pat scores (d=256) run DoubleRow over their two d-tiles.
  * Issue-order software pipelining tuned for the in-order engine
    queues: each branch drains its OWN V projection plus the next
    branch's Q/K projection inside the scores sweep, interleaved
    between score matmuls so the PE never head-of-line blocks on the
    scalar exp chain; g matmuls of head h-1 ride between the scores
    of head h.
  * fus1 and the MLP tail are kept in column form end-to-end: the
    o-proj+fus1 GEMV accumulates [128, 8] per branch via N=1 matmuls,
    so no row->column transposes and a short serial tail.
"""
import numpy as np
import ml_dtypes

import concourse.bass as bass
import concourse.tile as tile
from concourse import mybir
from concourse.bass_utils import run_bass_kernel_spmd

F32 = mybir.dt.float32
BF16 = mybir.dt.bfloat16
FP8 = mybir.dt.float8e4
AF = mybir.ActivationFunctionType
ALU = mybir.AluOpType
DR = mybir.MatmulPerfMode.DoubleRow
BF = ml_dtypes.bfloat16
E4 = ml_dtypes.float8_e4m3fn

P = 128
S = 1024
H = 1024
NT = H // P          # 8 tiles of 128 along H
NCORES = 8
MHAS = [("beh", 8), ("tmp", 4), ("pat", 4)]

SX = 32.0            # x fp8 scale
SW = 4096.0          # weight fp8 scale
DESC = 1.0 / (SX * SW)
SWP = float(2 ** 19)  # w' fp8 scale (undone after pooling)

_CACHE = {}


def _chunks(total, step=None):
    if step is None:
        step = -(-total // -(-total // 512))   # equal-ish chunks <= 512
    out = []
    o = 0
    while o < total:
        c = min(step, total - o)
        out.append((o, c))
        o += c
    return out


def _spread(n_units, n_heads):
    """Distribute n_units drain units over n_heads heads, front-loaded."""
    base = n_units // n_heads
    extra = n_units - base * n_heads
    return [base + (1 if h < extra else 0) for h in range(n_heads)]


def _build_nc(Sp, Sq):
    KT = Sp // P                 # key tiles
    QT = -(-Sq // P)             # query tiles (last may be partial)
    QL = Sq - (QT - 1) * P       # rows in last query tile
    NPAIR = NT // 2

    nc = bass.Bass()
    dram = {}

    def dp(name, shape, dt=FP8):
        dram[name] = nc.declare_dram_parameter(name, list(shape), dt,
                                               isOutput=False)

    dp("xT", (P, NT, Sp))
    dp("pw", (P, QT), F32)        # SWP/nv on valid queries, else 0
    dp("npadneg", (P, 1), F32)    # -(Sp - nv), replicated
    for m, _ in MHAS:
        for wn in ("qw", "kw", "vw"):
            dp(f"{m}_{wn}", (P, NT, H))
        dp(f"{m}_oww1", (P, NT, H), BF16)
        dp(f"{m}_qb", (P, NT), F32)
        dp(f"{m}_vb", (P, NT), F32)
    dp("b1col", (P, NT), F32)     # fus1_b + sum_m ob_m @ fus1_w[m]
    dp("fus2_w", (P, NT, H // 2), BF16)
    dp("b2r4", (1, 4, P), BF16)
    dp("cls_w", (P, 4, 2), BF16)
    dp("cbrow", (1, 2), F32)
    out = nc.declare_dram_parameter("out", [1, 2], F32, isOutput=True)

    with tile.TileContext(nc) as tc, \
         nc.allow_low_precision(
             reason="fp8 matmul operands with fp32 PSUM accumulation; "
                    "bias/pool math in fp32 on DVE"):
        with tc.tile_pool(name="big", bufs=1) as big, \
             tc.tile_pool(name="vbuf", bufs=2) as vbuf, \
             tc.tile_pool(name="wres", bufs=2) as wres, \
             tc.tile_pool(name="expp", bufs=2) as expp, \
             tc.tile_pool(name="small", bufs=1) as small, \
             tc.tile_pool(name="bias", bufs=3) as biasp, \
             tc.tile_pool(name="work", bufs=2) as work, \
             tc.tile_pool(name="pproj", bufs=2, space="PSUM") as pproj, \
             tc.tile_pool(name="psc", bufs=2, space="PSUM") as psc, \
             tc.tile_pool(name="pg", bufs=2, space="PSUM") as pgp:

            # ---- resident inputs ----
            xT = big.tile([P, NT, Sp], FP8, tag="xT")
            nc.sync.dma_start(xT[:], dram["xT"][:])
            QT8 = big.tile([P, NT, Sq], FP8, tag="QT8")
            KT8 = big.tile([P, NT, Sp], FP8, tag="KT8")

            pw = small.tile([P, QT], F32, tag="pw")
            nc.sync.dma_start(pw[:], dram["pw"][:])
            npadneg = small.tile([P, 1], F32, tag="npadneg")
            nc.sync.dma_start(npadneg[:], dram["npadneg"][:])

            h1acc = small.tile([P, NT], F32, tag="h1acc")
            nc.sync.dma_start(h1acc[:], dram["b1col"][:])

            def gated_dma(t, src):
                """DMA whose trigger waits for the DVE stream to reach
                this program point (WAW dep on a tiny memset), so bulk
                prefetches don't steal HBM bandwidth from earlier
                critical loads."""
                nc.vector.memset(t[0:1, 0:1, 0:1], 0.0)
                nc.sync.dma_start(t[:], src)

            def load_branch_weights(m, gated=False):
                ts = {}
                for wn in ("qw", "kw", "vw"):
                    t = wres.tile([P, NT, H], FP8, tag=f"w_{wn}",
                                  name=f"w_{m}_{wn}")
                    if gated:
                        gated_dma(t, dram[f"{m}_{wn}"][:])
                    else:
                        half = NT // 2
                        nc.sync.dma_start(t[:, 0:half],
                                          dram[f"{m}_{wn}"][:, 0:half])
                        nc.sync.dma_start(t[:, half:NT],
                                          dram[f"{m}_{wn}"][:, half:NT])
                    ts[wn] = t
                return ts

            # ---------------- projection emitters (generators) ----------
            def v_proj_chunks(wv, V):
                """Yield after each s-tile: V[st] = (x @ vw) * DESC, bf16."""
                for st in range(KT):
                    ssl = slice(st * P, (st + 1) * P)
                    pst = [pproj.tile([P, 512], F32, tag="pj",
                                      name=f"pjv_{id(V)}_{st}_{i}")
                           for i in range(2)]
                    for pr in range(NPAIR):
                        ksl = slice(2 * pr, 2 * pr + 2)
                        for hc in range(2):
                            hsl = slice(hc * 512, (hc + 1) * 512)
                            nc.tensor.matmul(
                                pst[hc][:], lhsT=xT[:, ksl, ssl],
                                rhs=wv[:, ksl, hsl],
                                start=(pr == 0), stop=(pr == NPAIR - 1),
                                perf_mode=DR)
                    for hc in range(2):
                        hsl = slice(hc * 512, (hc + 1) * 512)
                        nc.vector.tensor_scalar_mul(V[:, st, hsl],
                                                    pst[hc][:], DESC / SWP)
                    yield

            def qk_proj_chunks(wq, wk, qb):
                """Yield after each (weight, ho) unit. Q covers Sq query
                positions; K covers all Sp key positions."""
                for wn, wt, dst, tot in (("q", wq, QT8, Sq),
                                         ("k", wk, KT8, Sp)):
                    for ho in range(NT):
                        hsl = slice(ho * P, (ho + 1) * P)
                        cks = _chunks(tot)
                        pst = [pproj.tile([P, 512], F32, tag="pj",
                                          name=f"pj{wn}_{id(wt)}_{ho}_{i}")
                               for i in range(len(cks))]
                        for pr in range(NPAIR):
                            ksl = slice(2 * pr, 2 * pr + 2)
                            for ci, (o, w) in enumerate(cks):
                                nc.tensor.matmul(
                                    pst[ci][:, 0:w], lhsT=wt[:, ksl, hsl],
                                    rhs=xT[:, ksl, o:o + w],
                                    start=(pr == 0), stop=(pr == NPAIR - 1),
                                    perf_mode=DR)
                        for ci, (o, w) in enumerate(cks):
                            if wn == "q":
                                nc.vector.tensor_scalar(
                                    dst[:, ho, o:o + w], pst[ci][:, 0:w],
                                    scalar1=DESC, scalar2=qb[:, ho:ho + 1],
                                    op0=ALU.mult, op1=ALU.add)
                            else:
                                nc.vector.tensor_scalar_mul(
                                    dst[:, ho, o:o + w], pst[ci][:, 0:w],
                                    DESC)
                        yield

            def drain(gen, n=1):
                if gen is None:
                    return
                for _ in range(n):
                    try:
                        next(gen)
                    except StopIteration:
                        break

            def drain_all(gen):
                if gen is None:
                    return
                for _ in gen:
                    pass

            # ---------------- first branch Q/K projection ----------------
            m0, _ = MHAS[0]
            w_cur = {}
            for wn in ("qw", "kw"):
                t = wres.tile([P, NT, H], FP8, tag=f"w_{wn}",
                              name=f"w_{m0}_{wn}")
                half = NT // 2
                nc.sync.dma_start(t[:, 0:half], dram[f"{m0}_{wn}"][:, 0:half])
                nc.sync.dma_start(t[:, half:NT], dram[f"{m0}_{wn}"][:, half:NT])
                w_cur[wn] = t
            qb_t = biasp.tile([P, NT], F32, tag="qb", name="qb0")
            nc.sync.dma_start(qb_t[:], dram[f"{m0}_qb"][:])
            vb_t = biasp.tile([P, NT], F32, tag="vb", name="vb0")
            nc.sync.dma_start(vb_t[:], dram[f"{m0}_vb"][:])
            # PE warm-up while the prologue DMAs land: keeps the HAM
            # clock-gate at full rate so the first real matmuls run warm
            wrm = small.tile([P, 512], FP8, tag="wrm")
            nc.vector.memset(wrm[:], 0.0)
            _warm_ct = [0]

            def warm(n=12):
                _warm_ct[0] += 1
                pw_t = pproj.tile([P, 512], F32, tag="pj",
                                  name=f"warm{_warm_ct[0]}")
                for i in range(n):
                    nc.tensor.matmul(pw_t[:], lhsT=wrm[:, 0:128],
                                     rhs=wrm[:], start=(i == 0),
                                     stop=(i == n - 1))

            warm(48)
            qk0 = qk_proj_chunks(w_cur["qw"], w_cur["kw"], qb_t)
            drain(qk0, 10)
            t = wres.tile([P, NT, H], FP8, tag="w_vw", name=f"w_{m0}_vw")
            gated_dma(t, dram[f"{m0}_vw"][:])
            w_cur["vw"] = t
            drain_all(qk0)

            oww1 = wres.tile([P, NT, H], BF16, tag="w_oww1", name="oww1_0")
            gated_dma(oww1, dram[f"{m0}_oww1"][:])
            f2r = small.tile([P, NT, H // 2], BF16, tag="f2r")
            gated_dma(f2r, dram["fus2_w"][:])
            clsr = small.tile([P, 4, 2], BF16, tag="clsr")
            nc.sync.dma_start(clsr[:], dram["cls_w"][:])
            b2r4 = small.tile([1, 4, P], BF16, tag="b2r4")
            nc.sync.dma_start(b2r4[:], dram["b2r4"][:])
            oneb = small.tile([1, 1], BF16, tag="oneb")
            nc.vector.memset(oneb[:], 1.0)
            cbrow = small.tile([1, 2], F32, tag="cbrow")
            nc.sync.dma_start(cbrow[:], dram["cbrow"][:])

            kcks = _chunks(Sp, 512)      # scores psum chunks: matmul
                                         # writes must not cross banks

            tail_gen = None              # previous branch's epilogue units

            for mi, (m, nh) in enumerate(MHAS):
                d = H // nh
                ndt = d // P
                inv_sqrt_d = 1.0 / float(np.sqrt(d))

                # this branch's V projection (drained inside the sweep)
                V_cur = vbuf.tile([P, KT, H], BF16, tag="V",
                                  name=f"V_{m}")
                v_gen = v_proj_chunks(w_cur["vw"], V_cur)

                # prefetch next branch weights + Q/K emitter
                qk_gen = None
                if mi + 1 < len(MHAS):
                    mn, _ = MHAS[mi + 1]
                    w_nxt = load_branch_weights(mn, gated=True)
                    qb_n = biasp.tile([P, NT], F32, tag="qb",
                                      name=f"qb{mi + 1}")
                    nc.sync.dma_start(qb_n[:], dram[f"{mn}_qb"][:])
                    vb_n = biasp.tile([P, NT], F32, tag="vb",
                                      name=f"vb{mi + 1}")
                    nc.sync.dma_start(vb_n[:], dram[f"{mn}_vb"][:])
                    qk_gen = qk_proj_chunks(w_nxt["qw"], w_nxt["kw"], qb_n)

                EXP = expp.tile([P, QT, 8, Sp], FP8, tag="expb",
                                name=f"EXP{mi}")
                den = work.tile([P, QT, 8], F32, tag="den",
                                name=f"den{mi}")
                nc.vector.memset(den[:], 1.0)
                wp8 = work.tile([P, QT, 8], FP8, tag="wp8",
                                name=f"wp8{mi}")
                GC = KT * 8
                pg = pgp.tile([P, GC + 16], F32, tag="g", name=f"pg{mi}")
                G16 = work.tile([P, KT, 8], BF16, tag="G16",
                                name=f"G16{mi}")

                # front-loaded V drain so pooled can pipeline per head
                vplan = _spread(KT, min(nh, 2))
                vplan += [0] * (nh - len(vplan))
                qkplan = _spread(2 * NT, nh)

                def head_rec(h):
                    rec = work.tile([P, QT], F32, tag="rec",
                                    name=f"rec{mi}_{h}")
                    nc.vector.tensor_scalar_add(
                        rec[:], den[:, :, h], npadneg[:, 0:1])
                    nc.vector.reciprocal(rec[:], rec[:])
                    nc.vector.tensor_tensor(
                        out=wp8[:, :, h], in0=rec[:], in1=pw[:],
                        op=ALU.mult)

                def head_g(h):
                    for kt in range(KT):
                        for qt in range(QT):
                            qn = P if qt < QT - 1 else QL
                            nc.tensor.matmul(
                                pg[:, kt * 8 + h:kt * 8 + h + 1],
                                lhsT=EXP[0:qn, qt, h,
                                         kt * P:(kt + 1) * P],
                                rhs=wp8[0:qn, qt, h:h + 1],
                                start=(qt == 0), stop=(qt == QT - 1))

                def head_pooled(h, _pg=pg, _G16=G16, _V=V_cur,
                                _ndt=ndt, _GC=GC):
                    """G16 slice + pooled columns owned by head h."""
                    nc.vector.tensor_copy(
                        _G16[:, :, h],
                        _pg[:, 0:_GC].rearrange("p (k h) -> p k h",
                                                k=KT)[:, :, h])
                    for dc in range(h * _ndt, (h + 1) * _ndt):
                        for kt in range(KT):
                            nc.tensor.matmul(
                                _pg[:, _GC + dc:_GC + dc + 1],
                                lhsT=_V[:, kt, dc * P:(dc + 1) * P],
                                rhs=_G16[:, kt, h:h + 1],
                                start=(kt == 0), stop=(kt == KT - 1))

                def branch_tail(vb_cur, oww1_cur, pooled_left, _mi=mi,
                                _pg=pg, _GC=GC, _hp=head_pooled):
                    """Epilogue units; yields between PE clumps."""
                    for h in pooled_left:
                        _hp(h)
                    yield
                    pb = work.tile([P, NT], BF16, tag="poolb",
                                   name=f"pb{_mi}")
                    nc.vector.tensor_tensor(out=pb[:], in0=_pg[:, _GC:_GC + 8],
                                            in1=vb_cur[:], op=ALU.add)
                    for oc in range(NT):
                        for kt in range(NT):
                            nc.tensor.matmul(
                                _pg[:, _GC + 8 + oc:_GC + 9 + oc],
                                lhsT=oww1_cur[:, kt, oc * P:(oc + 1) * P],
                                rhs=pb[:, kt:kt + 1],
                                start=(kt == 0), stop=(kt == NT - 1))
                    yield
                    nc.vector.tensor_add(out=h1acc[:], in0=h1acc[:],
                                         in1=_pg[:, _GC + 8:_GC + 16])
                    yield

                # ---- per-head pipelined sweep ---------------------------
                # aux queue: previous branch's epilogue, then next
                # branch's Q/K projection, drained across fixed slots
                import itertools
                aux_srcs = [g for g in (tail_gen, qk_gen) if g is not None]
                aux = itertools.chain(*aux_srcs) if aux_srcs else None
                n_units = (3 if tail_gen is not None else 0) + \
                          (2 * NT if qk_gen is not None else 0)
                slots = []
                for h in range(nh):
                    for qt in range(QT):
                        if qt == 1 and h == 0:
                            slots.append((h, qt))
                        elif qt not in (0, 1) and not (qt == 3 and h >= 2):
                            slots.append((h, qt))
                auxplan = {hq: n for hq, n in
                           zip(slots, _spread(n_units, max(1, len(slots))))}
                for h in range(nh):
                    for qt in range(QT):
                        qn = P if qt < QT - 1 else QL
                        qsl = slice(qt * P, qt * P + qn)
                        sc = psc.tile([P, Sp], F32, tag="sc",
                                      name=f"sc{mi}_{h}_{qt}")
                        for (o, w) in kcks:
                            if ndt == 2:
                                nc.tensor.matmul(
                                    sc[0:qn, o:o + w],
                                    lhsT=QT8[:, 2 * h:2 * h + 2, qsl],
                                    rhs=KT8[:, 2 * h:2 * h + 2, o:o + w],
                                    start=True, stop=True, perf_mode=DR)
                            else:
                                nc.tensor.matmul(
                                    sc[0:qn, o:o + w],
                                    lhsT=QT8[:, h, qsl],
                                    rhs=KT8[:, h, o:o + w],
                                    start=True, stop=True)
                        nc.scalar.activation(
                            EXP[0:qn, qt, h, :], sc[0:qn, :], AF.Exp,
                            scale=inv_sqrt_d,
                            accum_out=den[0:qn, qt, h:h + 1])
                        # interleave aux PE work between score tiles
                        if qt == 0:
                            if h > 0:
                                head_rec(h - 1)
                            drain(v_gen, vplan[h])
                        elif qt == 1 and h > 0:
                            head_g(h - 1)
                        elif qt == 3 and h >= 2:
                            head_pooled(h - 2)
                        else:
                            drain(aux, auxplan.get((h, qt), 0))
                head_rec(nh - 1)
                head_g(nh - 1)
                if mi + 1 == len(MHAS):
                    warm()
                drain_all(v_gen)
                drain_all(aux)

                pooled_left = [nh - 2, nh - 1] if nh >= 2 else [0]
                tail_gen = branch_tail(vb_t, oww1, pooled_left)
                if mi + 1 == len(MHAS):
                    drain(tail_gen, 1)
                    warm()
                    drain(tail_gen, 1)
                    warm()
                    drain_all(tail_gen)
                    tail_gen = None

                # rotate per-branch state
                if mi + 1 < len(MHAS):
                    w_cur = w_nxt
                    qb_t, vb_t = qb_n, vb_n
                    mn, _ = MHAS[mi + 1]
                    oww1 = wres.tile([P, NT, H], BF16, tag="w_oww1",
                                     name=f"oww1_{mn}")
                    gated_dma(oww1, dram[f"{mn}_oww1"][:])

            # ---------------- MLP tail (all column-form) ------------------
            h1rc = small.tile([P, NT], BF16, tag="h1rc")
            nc.vector.tensor_relu(h1rc[:], h1acc[:])
            warm()
            tpg = pgp.tile([P, 4], F32, tag="g", name="tailpg")
            for oc in range(4):
                nc.tensor.matmul(tpg[:, oc:oc + 1],
                                 lhsT=b2r4[0:1, oc, :], rhs=oneb[:],
                                 start=True, stop=False)
                for kt in range(NT):
                    nc.tensor.matmul(
                        tpg[:, oc:oc + 1],
                        lhsT=f2r[:, kt, oc * P:(oc + 1) * P],
                        rhs=h1rc[:, kt:kt + 1],
                        start=False, stop=(kt == NT - 1))
            h2rc = small.tile([P, 4], BF16, tag="h2rc")
            nc.vector.tensor_relu(h2rc[:], tpg[:, 0:4])
            warm()

            plg = psc.tile([1, 2], F32, tag="sc", name="lg")
            for kt in range(4):
                nc.tensor.matmul(plg[:], lhsT=h2rc[:, kt:kt + 1],
                                 rhs=clsr[:, kt], start=(kt == 0),
                                 stop=(kt == 3))
            lg = small.tile([1, 2], F32, tag="lgsb")
            nc.vector.tensor_add(out=lg[:], in0=plg[:], in1=cbrow[:])
            nc.sync.dma_start(out[:], lg[:])

    _split_multi_waits(nc)
    return nc


def _split_multi_waits(nc, max_on_inst=1, max_on_evsem=2):
    """This walrus build caps sync waits per instruction at 1 (2 for
    EventSemaphore); Tile attaches one wait per dependent proc. Spill excess
    waits onto pure-wait EventSemaphores inserted before, on the same engine -
    the engine blocks on each condition in sequence, so semantics match."""
    for f in nc.m.functions:
        for bb in f.blocks:
            insts = list(bb.instructions)
            new = []
            changed = False
            for ins in insts:
                si = ins.sync_info
                if si is not None:
                    waits = list(si.on_wait)
                    cap = (max_on_evsem
                           if isinstance(ins, mybir.InstEventSemaphore)
                           else max_on_inst)
                    if len(waits) > cap:
                        spill = waits[:-cap]
                        keep = waits[-cap:]
                        k = 0
                        while spill:
                            chunk = spill[:max_on_evsem]
                            spill = spill[max_on_evsem:]
                            new.append(mybir.InstEventSemaphore(
                                name=f"{ins.name}-wspill{k}",
                                engine=ins.engine, ins=[], outs=[],
                                sync_info=mybir.SyncInfo(on_wait=chunk,
                                                         on_update=[])))
                            k += 1
                        ins.sync_info = mybir.SyncInfo(
                            on_wait=keep, on_update=list(si.on_update))
                        changed = True
                new.append(ins)
            if changed:
                bb.instructions = new


def _get_nc(Sp, Sq):
    if (Sp, Sq) not in _CACHE:
        _CACHE[(Sp, Sq)] = _build_nc(Sp, Sq)
    return _CACHE[(Sp, Sq)]


def _q8(a, scale):
    return np.clip(a.astype(np.float32) * scale, -240.0, 240.0).astype(E4)


def _h3(a):
    """[K, N] -> [P, K//P, N] partition-inner, contiguous."""
    K, N = a.shape
    return np.ascontiguousarray(a.reshape(K // P, P, N).transpose(1, 0, 2))


def _prep_in_maps(inputs, Sp, Sq):
    f32 = np.float32
    QT = -(-Sq // P)
    mask = inputs["attention_mask"].astype(np.int64)     # [B, S]

    w1 = inputs["fus1_w"].astype(f32)                    # [3H, H]
    shared = {
        "b2r4": np.ascontiguousarray(
            inputs["fus2_b"].astype(BF).reshape(1, 4, P)),
        "cbrow": inputs["cls_b"].astype(f32).reshape(1, 2),
        "fus2_w": _h3(inputs["fus2_w"].astype(BF)),
        "cls_w": _h3(inputs["cls_w"].astype(BF)),
    }

    b1 = inputs["fus1_b"].astype(f32).copy()
    for mi, (m, _) in enumerate(MHAS):
        w1m = w1[mi * H:(mi + 1) * H]                    # [H, H]
        for wn in ("qw", "kw", "vw"):
            shared[f"{m}_{wn}"] = _h3(_q8(inputs[f"{m}_{wn}"], SW))
        oww1 = inputs[f"{m}_ow"].astype(f32) @ w1m
        shared[f"{m}_oww1"] = _h3(oww1.astype(BF))
        b1 += inputs[f"{m}_ob"].astype(f32) @ w1m
        shared[f"{m}_qb"] = np.ascontiguousarray(
            inputs[f"{m}_qb"].astype(f32).reshape(NT, P).T)
        shared[f"{m}_vb"] = np.ascontiguousarray(
            inputs[f"{m}_vb"].astype(f32).reshape(NT, P).T)
    shared["b1col"] = np.ascontiguousarray(b1.reshape(NT, P).T)

    in_maps = []
    for c in range(NCORES):
        im = dict(shared)
        idx = np.nonzero(mask[c])[0]
        nv = len(idx)
        xp = np.zeros((Sp, H), f32)
        xp[:nv] = inputs["hidden_states"][c][idx]
        im["xT"] = _h3(_q8(xp.T, SX))
        pwv = np.zeros(QT * P, f32)
        pwv[:nv] = SWP / nv
        im["pw"] = np.ascontiguousarray(
            pwv.reshape(QT, P).T.astype(f32))
        im["npadneg"] = np.full((P, 1), -(Sp - nv), f32)
        in_maps.append(im)
    return in_maps


def kernel(**inputs) -> np.ndarray:
    mask = inputs["attention_mask"]
    maxc = int(mask.astype(np.int64).sum(axis=1).max())
    Sp = min(S, max(P, -(-maxc // P) * P))
    Sq = min(Sp, max(64, -(-maxc // 64) * 64))
    nc = _get_nc(Sp, Sq)
    in_maps = _prep_in_maps(inputs, Sp, Sq)
    res = run_bass_kernel_spmd(nc, in_maps, core_ids=list(range(NCORES)))
    return np.concatenate(
        [res.results[c]["out"] for c in range(NCORES)], axis=0
    ).astype(np.float32)


# revision 25
# speedup vs baseline: 1.0033x; 1.0033x over previous
"""EngagementPredictor TRN2 kernel: 3-branch MHA + masked mean-pool + MLP.

Sharding: pure data-parallel - B=8 batch elements, one per NeuronCore;
weights replicated; no collectives. Each core computes its [2]-logit row.

v3 design (over the bf16 baseline):
  * Valid-token packing as before (Sp keys / Sq queries, zero-padded).
  * fp8(e4m3) DoubleRow matmuls for all Q/K/V projections: weights are
    host-quantized at x4096, x at x32, contraction pairs two 128-row
    k-tiles per instruction -> half the PE stream cycles.
  * Scores in [query-part, key-free] orientation (lhsT=Q). The exp
    activation produces the softmax denominator for free via accum_out
    (free-axis sum); padded keys have x=0 -> score 0 -> exp=1, their
    count is subtracted from the denominator and their V rows are
    exactly zero, so no mask tensor is needed at all.
  * Pooling: pooled[d] = sum_k V[k,d] * g[k], g[k] = sum_q exp[q,k] *
    (poolw[q]/den[q]); g and pooled are N=1 matmuls (no ctx
    materialization, no broadcast, no big DVE reductions).
  * tmp/pat scores (d=256) run DoubleRow over their two d-tiles.
  * Issue-order software pipelining tuned for the in-order engine
    queues: each branch drains its OWN V projection plus the next
    branch's Q/K projection inside the scores sweep, interleaved
    between score matmuls so the PE never head-of-line blocks on the
    scalar exp chain; g matmuls of head h-1 ride between the scores
    of head h.
  * fus1 and the MLP tail are kept in column form end-to-end: the
    o-proj+fus1 GEMV accumulates [128, 8] per branch via N=1 matmuls,
    so no row->column transposes and a short serial tail.
"""
import numpy as np
import ml_dtypes

import concourse.bass as bass
import concourse.tile as tile
from concourse import mybir
from concourse.bass_utils import run_bass_kernel_spmd

F32 = mybir.dt.float32
BF16 = mybir.dt.bfloat16
FP8 = mybir.dt.float8e4
AF = mybir.ActivationFunctionType
ALU = mybir.AluOpType
DR = mybir.MatmulPerfMode.DoubleRow
BF = ml_dtypes.bfloat16
E4 = ml_dtypes.float8_e4m3fn

P = 128
S = 1024
H = 1024
NT = H // P          # 8 tiles of 128 along H
NCORES = 8
MHAS = [("beh", 8), ("tmp", 4), ("pat", 4)]

SX = 32.0            # x fp8 scale
SW = 4096.0          # weight fp8 scale
DESC = 1.0 / (SX * SW)
SWP = float(2 ** 19)  # w' fp8 scale (undone after pooling)

_CACHE = {}


def _chunks(total, step=None):
    if step is None:
        step = -(-total // -(-total // 512))   # equal-ish chunks <= 512
    out = []
    o = 0
    while o < total:
        c = min(step, total - o)
        out.append((o, c))
        o += c
    return out


def _spread(n_units, n_heads):
    """Distribute n_units drain units over n_heads heads, front-loaded."""
    base = n_units // n_heads
    extra = n_units - base * n_heads
    return [base + (1 if h < extra else 0) for h in range(n_heads)]


def _build_nc(Sp, Sq):
    KT = Sp // P                 # key tiles
    QT = -(-Sq // P)             # query tiles (last may be partial)
    QL = Sq - (QT - 1) * P       # rows in last query tile
    NPAIR = NT // 2

    nc = bass.Bass()
    dram = {}

    def dp(name, shape, dt=FP8):
        dram[name] = nc.declare_dram_parameter(name, list(shape), dt,
                                               isOutput=False)

    dp("xT", (P, NT, Sp))
    dp("pw", (P, QT), F32)        # SWP/nv on valid queries, else 0
    dp("npadneg", (P, 1), F32)    # -(Sp - nv), replicated
    for m, _ in MHAS:
        for wn in ("qw", "kw", "vw"):
            dp(f"{m}_{wn}", (P, NT, H))
        dp(f"{m}_oww1", (P, NT, H), BF16)
        dp(f"{m}_qb", (P, NT), F32)
        dp(f"{m}_vb", (P, NT), F32)
    dp("b1col", (P, NT), F32)     # fus1_b + sum_m ob_m @ fus1_w[m]
    dp("fus2_w", (P, NT, H // 2), BF16)
    dp("b2r4", (1, 4, P), BF16)
    dp("cls_w", (P, 4, 2), BF16)
    dp("cbrow", (1, 2), F32)
    out = nc.declare_dram_parameter("out", [1, 2], F32, isOutput=True)

    with tile.TileContext(nc) as tc, \
         nc.allow_low_precision(
             reason="fp8 matmul operands with fp32 PSUM accumulation; "
                    "bias/pool math in fp32 on DVE"):
        with tc.tile_pool(name="big", bufs=1) as big, \
             tc.tile_pool(name="vbuf", bufs=2) as vbuf, \
             tc.tile_pool(name="wres", bufs=2) as wres, \
             tc.tile_pool(name="expp", bufs=2) as expp, \
             tc.tile_pool(name="small", bufs=1) as small, \
             tc.tile_pool(name="bias", bufs=3) as biasp, \
             tc.tile_pool(name="work", bufs=2) as work, \
             tc.tile_pool(name="pproj", bufs=2, space="PSUM") as pproj, \
             tc.tile_pool(name="psc", bufs=2, space="PSUM") as psc, \
             tc.tile_pool(name="pg", bufs=2, space="PSUM") as pgp:

            # ---- resident inputs ----
            xT = big.tile([P, NT, Sp], FP8, tag="xT")
            nc.sync.dma_start(xT[:], dram["xT"][:])
            QT8 = big.tile([P, NT, Sq], FP8, tag="QT8")
            KT8 = big.tile([P, NT, Sp], FP8, tag="KT8")

            pw = small.tile([P, QT], F32, tag="pw")
            nc.sync.dma_start(pw[:], dram["pw"][:])
            npadneg = small.tile([P, 1], F32, tag="npadneg")
            nc.sync.dma_start(npadneg[:], dram["npadneg"][:])

            h1acc = small.tile([P, NT], F32, tag="h1acc")
            nc.sync.dma_start(h1acc[:], dram["b1col"][:])

            def gated_dma(t, src):
                """DMA whose trigger waits for the DVE stream to reach
                this program point (WAW dep on a tiny memset), so bulk
                prefetches don't steal HBM bandwidth from earlier
                critical loads."""
                nc.vector.memset(t[0:1, 0:1, 0:1], 0.0)
                nc.sync.dma_start(t[:], src)

            def load_branch_weights(m, gated=False):
                ts = {}
                for wn in ("qw", "kw", "vw"):
                    t = wres.tile([P, NT, H], FP8, tag=f"w_{wn}",
                                  name=f"w_{m}_{wn}")
                    if gated:
                        gated_dma(t, dram[f"{m}_{wn}"][:])
                    else:
                        half = NT // 2
                        nc.sync.dma_start(t[:, 0:half],
                                          dram[f"{m}_{wn}"][:, 0:half])
                        nc.sync.dma_start(t[:, half:NT],
                                          dram[f"{m}_{wn}"][:, half:NT])
                    ts[wn] = t
                return ts

            # ---------------- projection emitters (generators) ----------
            def v_proj_chunks(wv, V):
                """Yield after each s-tile: V[st] = (x @ vw) * DESC, bf16."""
                for st in range(KT):
                    ssl = slice(st * P, (st + 1) * P)
                    pst = [pproj.tile([P, 512], F32, tag="pj",
                                      name=f"pjv_{id(V)}_{st}_{i}")
                           for i in range(2)]
                    for pr in range(NPAIR):
                        ksl = slice(2 * pr, 2 * pr + 2)
                        for hc in range(2):
                            hsl = slice(hc * 512, (hc + 1) * 512)
                            nc.tensor.matmul(
                                pst[hc][:], lhsT=xT[:, ksl, ssl],
                                rhs=wv[:, ksl, hsl],
                                start=(pr == 0), stop=(pr == NPAIR - 1),
                                perf_mode=DR)
                    for hc in range(2):
                        hsl = slice(hc * 512, (hc + 1) * 512)
                        nc.vector.tensor_scalar_mul(V[:, st, hsl],
                                                    pst[hc][:], DESC / SWP)
                    yield

            def qk_proj_chunks(wq, wk, qb):
                """Yield after each (weight, ho) unit. Q covers Sq query
                positions; K covers all Sp key positions."""
                for wn, wt, dst, tot in (("q", wq, QT8, Sq),
                                         ("k", wk, KT8, Sp)):
                    for ho in range(NT):
                        hsl = slice(ho * P, (ho + 1) * P)
                        cks = _chunks(tot)
                        pst = [pproj.tile([P, 512], F32, tag="pj",
                                          name=f"pj{wn}_{id(wt)}_{ho}_{i}")
                               for i in range(len(cks))]
                        for pr in range(NPAIR):
                            ksl = slice(2 * pr, 2 * pr + 2)
                            for ci, (o, w) in enumerate(cks):
                                nc.tensor.matmul(
                                    pst[ci][:, 0:w], lhsT=wt[:, ksl, hsl],
                                    rhs=xT[:, ksl, o:o + w],
                                    start=(pr == 0), stop=(pr == NPAIR - 1),
                                    perf_mode=DR)
                        for ci, (o, w) in enumerate(cks):
                            if wn == "q":
                                nc.vector.tensor_scalar(
                                    dst[:, ho, o:o + w], pst[ci][:, 0:w],
                                    scalar1=DESC, scalar2=qb[:, ho:ho + 1],
                                    op0=ALU.mult, op1=ALU.add)
                            else:
                                nc.vector.tensor_scalar_mul(
                                    dst[:, ho, o:o + w], pst[ci][:, 0:w],
                                    DESC)
                        yield

            def drain(gen, n=1):
                if gen is None:
                    return
                for _ in range(n):
                    try:
                        next(gen)
                    except StopIteration:
                        break

            def drain_all(gen):
                if gen is None:
                    return
                for _ in gen:
                    pass

            # ---------------- first branch Q/K projection ----------------
            m0, _ = MHAS[0]
            w_cur = {}
            for wn in ("qw", "kw"):
                t = wres.tile([P, NT, H], FP8, tag=f"w_{wn}",
                              name=f"w_{m0}_{wn}")
                half = NT // 2
                nc.sync.dma_start(t[:, 0:half], dram[f"{m0}_{wn}"][:, 0:half])
                nc.sync.dma_start(t[:, half:NT], dram[f"{m0}_{wn}"][:, half:NT])
                w_cur[wn] = t
            qb_t = biasp.tile([P, NT], F32, tag="qb", name="qb0")
            nc.sync.dma_start(qb_t[:], dram[f"{m0}_qb"][:])
            vb_t = biasp.tile([P, NT], F32, tag="vb", name="vb0")
            nc.sync.dma_start(vb_t[:], dram[f"{m0}_vb"][:])
            # PE warm-up while the prologue DMAs land: keeps the HAM
            # clock-gate at full rate so the first real matmuls run warm
            wrm = small.tile([P, 512], FP8, tag="wrm")
            nc.vector.memset(wrm[:], 0.0)
            _warm_ct = [0]

            def warm(n=6):
                _warm_ct[0] += 1
                pw_t = pproj.tile([P, 512], F32, tag="pj",
                                  name=f"warm{_warm_ct[0]}")
                for i in range(n):
                    nc.tensor.matmul(pw_t[:], lhsT=wrm[:, 0:128],
                                     rhs=wrm[:], start=(i == 0),
                                     stop=(i == n - 1))

            warm(48)
            qk0 = qk_proj_chunks(w_cur["qw"], w_cur["kw"], qb_t)
            drain(qk0, 10)
            t = wres.tile([P, NT, H], FP8, tag="w_vw", name=f"w_{m0}_vw")
            gated_dma(t, dram[f"{m0}_vw"][:])
            w_cur["vw"] = t
            drain_all(qk0)

            oww1 = wres.tile([P, NT, H], BF16, tag="w_oww1", name="oww1_0")
            gated_dma(oww1, dram[f"{m0}_oww1"][:])
            f2r = small.tile([P, NT, H // 2], BF16, tag="f2r")
            gated_dma(f2r, dram["fus2_w"][:])
            clsr = small.tile([P, 4, 2], BF16, tag="clsr")
            nc.sync.dma_start(clsr[:], dram["cls_w"][:])
            b2r4 = small.tile([1, 4, P], BF16, tag="b2r4")
            nc.sync.dma_start(b2r4[:], dram["b2r4"][:])
            oneb = small.tile([1, 1], BF16, tag="oneb")
            nc.vector.memset(oneb[:], 1.0)
            cbrow = small.tile([1, 2], F32, tag="cbrow")
            nc.sync.dma_start(cbrow[:], dram["cbrow"][:])

            kcks = _chunks(Sp, 512)      # scores psum chunks: matmul
                                         # writes must not cross banks

            tail_gen = None              # previous branch's epilogue units

            for mi, (m, nh) in enumerate(MHAS):
                d = H // nh
                ndt = d // P
                inv_sqrt_d = 1.0 / float(np.sqrt(d))

                # this branch's V projection (drained inside the sweep)
                V_cur = vbuf.tile([P, KT, H], BF16, tag="V",
                                  name=f"V_{m}")
                v_gen = v_proj_chunks(w_cur["vw"], V_cur)

                # prefetch next branch weights + Q/K emitter
                qk_gen = None
                if mi + 1 < len(MHAS):
                    mn, _ = MHAS[mi + 1]
                    w_nxt = load_branch_weights(mn, gated=True)
                    qb_n = biasp.tile([P, NT], F32, tag="qb",
                                      name=f"qb{mi + 1}")
                    nc.sync.dma_start(qb_n[:], dram[f"{mn}_qb"][:])
                    vb_n = biasp.tile([P, NT], F32, tag="vb",
                                      name=f"vb{mi + 1}")
                    nc.sync.dma_start(vb_n[:], dram[f"{mn}_vb"][:])
                    qk_gen = qk_proj_chunks(w_nxt["qw"], w_nxt["kw"], qb_n)

                EXP = expp.tile([P, QT, 8, Sp], FP8, tag="expb",
                                name=f"EXP{mi}")
                den = work.tile([P, QT, 8], F32, tag="den",
                                name=f"den{mi}")
                nc.vector.memset(den[:], 1.0)
                wp8 = work.tile([P, QT, 8], FP8, tag="wp8",
                                name=f"wp8{mi}")
                GC = KT * 8
                pg = pgp.tile([P, GC + 16], F32, tag="g", name=f"pg{mi}")
                G16 = work.tile([P, KT, 8], BF16, tag="G16",
                                name=f"G16{mi}")

                # V drain spread over the first 3 heads: completes by
                # h2-qt0, just in time for the first pooled at h2-qt3,
                # without overloading heads 0-1
                vplan = _spread(KT, min(nh, 3))
                vplan += [0] * (nh - len(vplan))
                qkplan = _spread(2 * NT, nh)

                def head_rec(h):
                    rec = work.tile([P, QT], F32, tag="rec",
                                    name=f"rec{mi}_{h}")
                    nc.vector.tensor_scalar_add(
                        rec[:], den[:, :, h], npadneg[:, 0:1])
                    nc.vector.reciprocal(rec[:], rec[:])
                    nc.vector.tensor_tensor(
                        out=wp8[:, :, h], in0=rec[:], in1=pw[:],
                        op=ALU.mult)

                def head_g(h):
                    for kt in range(KT):
                        for qt in range(QT):
                            qn = P if qt < QT - 1 else QL
                            nc.tensor.matmul(
                                pg[:, kt * 8 + h:kt * 8 + h + 1],
                                lhsT=EXP[0:qn, qt, h,
                                         kt * P:(kt + 1) * P],
                                rhs=wp8[0:qn, qt, h:h + 1],
                                start=(qt == 0), stop=(qt == QT - 1))

                def head_pooled(h, _pg=pg, _G16=G16, _V=V_cur,
                                _ndt=ndt, _GC=GC):
                    """G16 slice + pooled columns owned by head h."""
                    nc.vector.tensor_copy(
                        _G16[:, :, h],
                        _pg[:, 0:_GC].rearrange("p (k h) -> p k h",
                                                k=KT)[:, :, h])
                    for dc in range(h * _ndt, (h + 1) * _ndt):
                        for kt in range(KT):
                            nc.tensor.matmul(
                                _pg[:, _GC + dc:_GC + dc + 1],
                                lhsT=_V[:, kt, dc * P:(dc + 1) * P],
                                rhs=_G16[:, kt, h:h + 1],
                                start=(kt == 0), stop=(kt == KT - 1))

                def branch_tail(vb_cur, oww1_cur, pooled_left, _mi=mi,
                                _pg=pg, _GC=GC, _hp=head_pooled):
                    """Epilogue units; yields between PE clumps."""
                    for h in pooled_left:
                        _hp(h)
                    yield
                    pb = work.tile([P, NT], BF16, tag="poolb",
                                   name=f"pb{_mi}")
                    nc.vector.tensor_tensor(out=pb[:], in0=_pg[:, _GC:_GC + 8],
                                            in1=vb_cur[:], op=ALU.add)
                    for oc in range(NT):
                        for kt in range(NT):
                            nc.tensor.matmul(
                                _pg[:, _GC + 8 + oc:_GC + 9 + oc],
                                lhsT=oww1_cur[:, kt, oc * P:(oc + 1) * P],
                                rhs=pb[:, kt:kt + 1],
                                start=(kt == 0), stop=(kt == NT - 1))
                    yield
                    nc.vector.tensor_add(out=h1acc[:], in0=h1acc[:],
                                         in1=_pg[:, _GC + 8:_GC + 16])
                    yield

                # ---- per-head pipelined sweep ---------------------------
                # aux queue: previous branch's epilogue, then next
                # branch's Q/K projection, drained across fixed slots
                import itertools
                aux_srcs = [g for g in (tail_gen, qk_gen) if g is not None]
                aux = itertools.chain(*aux_srcs) if aux_srcs else None
                n_units = (3 if tail_gen is not None else 0) + \
                          (2 * NT if qk_gen is not None else 0)
                slots = []
                for h in range(nh):
                    for qt in range(QT):
                        if qt == 1 and h == 0:
                            slots.append((h, qt))
                        elif qt not in (0, 1) and not (qt == 3 and h >= 2):
                            slots.append((h, qt))
                auxplan = {hq: n for hq, n in
                           zip(slots, _spread(n_units, max(1, len(slots))))}
                for h in range(nh):
                    for qt in range(QT):
                        qn = P if qt < QT - 1 else QL
                        qsl = slice(qt * P, qt * P + qn)
                        sc = psc.tile([P, Sp], F32, tag="sc",
                                      name=f"sc{mi}_{h}_{qt}")
                        for (o, w) in kcks:
                            if ndt == 2:
                                nc.tensor.matmul(
                                    sc[0:qn, o:o + w],
                                    lhsT=QT8[:, 2 * h:2 * h + 2, qsl],
                                    rhs=KT8[:, 2 * h:2 * h + 2, o:o + w],
                                    start=True, stop=True, perf_mode=DR)
                            else:
                                nc.tensor.matmul(
                                    sc[0:qn, o:o + w],
                                    lhsT=QT8[:, h, qsl],
                                    rhs=KT8[:, h, o:o + w],
                                    start=True, stop=True)
                        nc.scalar.activation(
                            EXP[0:qn, qt, h, :], sc[0:qn, :], AF.Exp,
                            scale=inv_sqrt_d,
                            accum_out=den[0:qn, qt, h:h + 1])
                        # interleave aux PE work between score tiles
                        if qt == 0:
                            if h > 0:
                                head_rec(h - 1)
                            drain(v_gen, vplan[h])
                        elif qt == 1 and h > 0:
                            head_g(h - 1)
                        elif qt == 3 and h >= 2:
                            head_pooled(h - 2)
                        else:
                            drain(aux, auxplan.get((h, qt), 0))
                head_rec(nh - 1)
                head_g(nh - 1)
                drain_all(v_gen)
                drain_all(aux)

                pooled_left = [nh - 2, nh - 1] if nh >= 2 else [0]
                tail_gen = branch_tail(vb_t, oww1, pooled_left)
                if mi + 1 == len(MHAS):
                    drain(tail_gen, 1)
                    warm()
                    drain(tail_gen, 1)
                    warm()
                    drain_all(tail_gen)
                    tail_gen = None

                # rotate per-branch state
                if mi + 1 < len(MHAS):
                    w_cur = w_nxt
                    qb_t, vb_t = qb_n, vb_n
                    mn, _ = MHAS[mi + 1]
                    oww1 = wres.tile([P, NT, H], BF16, tag="w_oww1",
                                     name=f"oww1_{mn}")
                    gated_dma(oww1, dram[f"{mn}_oww1"][:])

            # ---------------- MLP tail (all column-form) ------------------
            h1rc = small.tile([P, NT], BF16, tag="h1rc")
            nc.vector.tensor_relu(h1rc[:], h1acc[:])
            warm()
            tpg = pgp.tile([P, 4], F32, tag="g", name="tailpg")
            for oc in range(4):
                nc.tensor.matmul(tpg[:, oc:oc + 1],
                                 lhsT=b2r4[0:1, oc, :], rhs=oneb[:],
                                 start=True, stop=False)
                for kt in range(NT):
                    nc.tensor.matmul(
                        tpg[:, oc:oc + 1],
                        lhsT=f2r[:, kt, oc * P:(oc + 1) * P],
                        rhs=h1rc[:, kt:kt + 1],
                        start=False, stop=(kt == NT - 1))
            h2rc = small.tile([P, 4], BF16, tag="h2rc")
            nc.vector.tensor_relu(h2rc[:], tpg[:, 0:4])
            warm()

            plg = psc.tile([1, 2], F32, tag="sc", name="lg")
            for kt in range(4):
                nc.tensor.matmul(plg[:], lhsT=h2rc[:, kt:kt + 1],
                                 rhs=clsr[:, kt], start=(kt == 0),
                                 stop=(kt == 3))
            lg = small.tile([1, 2], F32, tag="lgsb")
            nc.vector.tensor_add(out=lg[:], in0=plg[:], in1=cbrow[:])
            nc.sync.dma_start(out[:], lg[:])

    _split_multi_waits(nc)
    return nc


def _split_multi_waits(nc, max_on_inst=1, max_on_evsem=2):
    """This walrus build caps sync waits per instruction at 1 (2 for
    EventSemaphore); Tile attaches one wait per dependent proc. Spill excess
    waits onto pure-wait EventSemaphores inserted before, on the same engine -
    the engine blocks on each condition in sequence, so semantics match."""
    for f in nc.m.functions:
        for bb in f.blocks:
            insts = list(bb.instructions)
            new = []
            changed = False
            for ins in insts:
                si = ins.sync_info
                if si is not None:
                    waits = list(si.on_wait)
                    cap = (max_on_evsem
                           if isinstance(ins, mybir.InstEventSemaphore)
                           else max_on_inst)
                    if len(waits) > cap:
                        spill = waits[:-cap]
                        keep = waits[-cap:]
                        k = 0
                        while spill:
                            chunk = spill[:max_on_evsem]
                            spill = spill[max_on_evsem:]
                            new.append(mybir.InstEventSemaphore(
                                name=f"{ins.name}-wspill{k}",
                                engine=ins.engine, ins=[], outs=[],
                                sync_info=mybir.SyncInfo(on_wait=chunk,
                                                         on_update=[])))
                            k += 1
                        ins.sync_info = mybir.SyncInfo(
                            on_wait=keep, on_update=list(si.on_update))
                        changed = True
                new.append(ins)
            if changed:
                bb.instructions = new


def _get_nc(Sp, Sq):
    if (Sp, Sq) not in _CACHE:
        _CACHE[(Sp, Sq)] = _build_nc(Sp, Sq)
    return _CACHE[(Sp, Sq)]


def _q8(a, scale):
    return np.clip(a.astype(np.float32) * scale, -240.0, 240.0).astype(E4)


def _h3(a):
    """[K, N] -> [P, K//P, N] partition-inner, contiguous."""
    K, N = a.shape
    return np.ascontiguousarray(a.reshape(K // P, P, N).transpose(1, 0, 2))


def _prep_in_maps(inputs, Sp, Sq):
    f32 = np.float32
    QT = -(-Sq // P)
    mask = inputs["attention_mask"].astype(np.int64)     # [B, S]

    w1 = inputs["fus1_w"].astype(f32)                    # [3H, H]
    shared = {
        "b2r4": np.ascontiguousarray(
            inputs["fus2_b"].astype(BF).reshape(1, 4, P)),
        "cbrow": inputs["cls_b"].astype(f32).reshape(1, 2),
        "fus2_w": _h3(inputs["fus2_w"].astype(BF)),
        "cls_w": _h3(inputs["cls_w"].astype(BF)),
    }

    b1 = inputs["fus1_b"].astype(f32).copy()
    for mi, (m, _) in enumerate(MHAS):
        w1m = w1[mi * H:(mi + 1) * H]                    # [H, H]
        for wn in ("qw", "kw", "vw"):
            shared[f"{m}_{wn}"] = _h3(_q8(inputs[f"{m}_{wn}"], SW))
        oww1 = inputs[f"{m}_ow"].astype(f32) @ w1m
        shared[f"{m}_oww1"] = _h3(oww1.astype(BF))
        b1 += inputs[f"{m}_ob"].astype(f32) @ w1m
        shared[f"{m}_qb"] = np.ascontiguousarray(
            inputs[f"{m}_qb"].astype(f32).reshape(NT, P).T)
        shared[f"{m}_vb"] = np.ascontiguousarray(
            inputs[f"{m}_vb"].astype(f32).reshape(NT, P).T)
    shared["b1col"] = np.ascontiguousarray(b1.reshape(NT, P).T)

    in_maps = []
    for c in range(NCORES):
        im = dict(shared)
        idx = np.nonzero(mask[c])[0]
        nv = len(idx)
        xp = np.zeros((Sp, H), f32)
        xp[:nv] = inputs["hidden_states"][c][idx]
        im["xT"] = _h3(_q8(xp.T, SX))
        pwv = np.zeros(QT * P, f32)
        pwv[:nv] = SWP / nv
        im["pw"] = np.ascontiguousarray(
            pwv.reshape(QT, P).T.astype(f32))
        im["npadneg"] = np.full((P, 1), -(Sp - nv), f32)
        in_maps.append(im)
    return in_maps


def kernel(**inputs) -> np.ndarray:
    mask = inputs["attention_mask"]
    maxc = int(mask.astype(np.int64).sum(axis=1).max())
    Sp = min(S, max(P, -(-maxc // P) * P))
    Sq = min(Sp, max(64, -(-maxc // 64) * 64))
    nc = _get_nc(Sp, Sq)
    in_maps = _prep_in_maps(inputs, Sp, Sq)
    res = run_bass_kernel_spmd(nc, in_maps, core_ids=list(range(NCORES)))
    return np.concatenate(
        [res.results[c]["out"] for c in range(NCORES)], axis=0
    ).astype(np.float32)


# revision 27
# speedup vs baseline: 1.0035x; 1.0002x over previous
"""EngagementPredictor TRN2 kernel: 3-branch MHA + masked mean-pool + MLP.

Sharding: pure data-parallel - B=8 batch elements, one per NeuronCore;
weights replicated; no collectives. Each core computes its [2]-logit row.

v3 design (over the bf16 baseline):
  * Valid-token packing as before (Sp keys / Sq queries, zero-padded).
  * fp8(e4m3) DoubleRow matmuls for all Q/K/V projections: weights are
    host-quantized at x4096, x at x32, contraction pairs two 128-row
    k-tiles per instruction -> half the PE stream cycles.
  * Scores in [query-part, key-free] orientation (lhsT=Q). The exp
    activation produces the softmax denominator for free via accum_out
    (free-axis sum); padded keys have x=0 -> score 0 -> exp=1, their
    count is subtracted from the denominator and their V rows are
    exactly zero, so no mask tensor is needed at all.
  * Pooling: pooled[d] = sum_k V[k,d] * g[k], g[k] = sum_q exp[q,k] *
    (poolw[q]/den[q]); g and pooled are N=1 matmuls (no ctx
    materialization, no broadcast, no big DVE reductions).
  * tmp/pat scores (d=256) run DoubleRow over their two d-tiles.
  * Issue-order software pipelining tuned for the in-order engine
    queues: each branch drains its OWN V projection plus the next
    branch's Q/K projection inside the scores sweep, interleaved
    between score matmuls so the PE never head-of-line blocks on the
    scalar exp chain; g matmuls of head h-1 ride between the scores
    of head h.
  * fus1 and the MLP tail are kept in column form end-to-end: the
    o-proj+fus1 GEMV accumulates [128, 8] per branch via N=1 matmuls,
    so no row->column transposes and a short serial tail.
"""
import numpy as np
import ml_dtypes

import concourse.bass as bass
import concourse.tile as tile
from concourse import mybir
from concourse.bass_utils import run_bass_kernel_spmd

F32 = mybir.dt.float32
BF16 = mybir.dt.bfloat16
FP8 = mybir.dt.float8e4
AF = mybir.ActivationFunctionType
ALU = mybir.AluOpType
DR = mybir.MatmulPerfMode.DoubleRow
BF = ml_dtypes.bfloat16
E4 = ml_dtypes.float8_e4m3fn

P = 128
S = 1024
H = 1024
NT = H // P          # 8 tiles of 128 along H
NCORES = 8
MHAS = [("beh", 8), ("tmp", 4), ("pat", 4)]

SX = 32.0            # x fp8 scale
SW = 4096.0          # weight fp8 scale
DESC = 1.0 / (SX * SW)
SWP = float(2 ** 19)  # w' fp8 scale (undone after pooling)

_CACHE = {}


def _chunks(total, step=None):
    if step is None:
        step = -(-total // -(-total // 512))   # equal-ish chunks <= 512
    out = []
    o = 0
    while o < total:
        c = min(step, total - o)
        out.append((o, c))
        o += c
    return out


def _spread(n_units, n_heads):
    """Distribute n_units drain units over n_heads heads, front-loaded."""
    base = n_units // n_heads
    extra = n_units - base * n_heads
    return [base + (1 if h < extra else 0) for h in range(n_heads)]


def _build_nc(Sp, Sq):
    KT = Sp // P                 # key tiles
    QT = -(-Sq // P)             # query tiles (last may be partial)
    QL = Sq - (QT - 1) * P       # rows in last query tile
    NPAIR = NT // 2

    nc = bass.Bass()
    dram = {}

    def dp(name, shape, dt=FP8):
        dram[name] = nc.declare_dram_parameter(name, list(shape), dt,
                                               isOutput=False)

    dp("xT", (P, NT, Sp))
    dp("pw", (P, QT), F32)        # SWP/nv on valid queries, else 0
    dp("npadneg", (P, 1), F32)    # -(Sp - nv), replicated
    for m, _ in MHAS:
        for wn in ("qw", "kw", "vw"):
            dp(f"{m}_{wn}", (P, NT, H))
        dp(f"{m}_oww1", (P, NT, H), BF16)
        dp(f"{m}_qb", (P, NT), F32)
        dp(f"{m}_vb", (P, NT), F32)
    dp("b1col", (P, NT), F32)     # fus1_b + sum_m ob_m @ fus1_w[m]
    dp("fus2_w", (P, NT, H // 2), BF16)
    dp("b2r4", (1, 4, P), BF16)
    dp("cls_w", (P, 4, 2), BF16)
    dp("cbrow", (1, 2), F32)
    out = nc.declare_dram_parameter("out", [1, 2], F32, isOutput=True)

    with tile.TileContext(nc) as tc, \
         nc.allow_low_precision(
             reason="fp8 matmul operands with fp32 PSUM accumulation; "
                    "bias/pool math in fp32 on DVE"):
        with tc.tile_pool(name="big", bufs=1) as big, \
             tc.tile_pool(name="vbuf", bufs=2) as vbuf, \
             tc.tile_pool(name="wres", bufs=2) as wres, \
             tc.tile_pool(name="expp", bufs=2) as expp, \
             tc.tile_pool(name="small", bufs=1) as small, \
             tc.tile_pool(name="bias", bufs=3) as biasp, \
             tc.tile_pool(name="work", bufs=2) as work, \
             tc.tile_pool(name="pproj", bufs=2, space="PSUM") as pproj, \
             tc.tile_pool(name="psc", bufs=2, space="PSUM") as psc, \
             tc.tile_pool(name="pg", bufs=2, space="PSUM") as pgp:

            # ---- resident inputs ----
            xT = big.tile([P, NT, Sp], FP8, tag="xT")
            nc.sync.dma_start(xT[:], dram["xT"][:])
            QT8 = big.tile([P, NT, Sq], FP8, tag="QT8")
            KT8 = big.tile([P, NT, Sp], FP8, tag="KT8")

            pw = small.tile([P, QT], F32, tag="pw")
            nc.sync.dma_start(pw[:], dram["pw"][:])
            npadneg = small.tile([P, 1], F32, tag="npadneg")
            nc.sync.dma_start(npadneg[:], dram["npadneg"][:])

            h1acc = small.tile([P, NT], F32, tag="h1acc")
            nc.sync.dma_start(h1acc[:], dram["b1col"][:])

            def gated_dma(t, src):
                """DMA whose trigger waits for the DVE stream to reach
                this program point (WAW dep on a tiny memset), so bulk
                prefetches don't steal HBM bandwidth from earlier
                critical loads."""
                nc.vector.memset(t[0:1, 0:1, 0:1], 0.0)
                nc.sync.dma_start(t[:], src)

            def load_branch_weights(m, gated=False):
                ts = {}
                for wn in ("qw", "kw", "vw"):
                    t = wres.tile([P, NT, H], FP8, tag=f"w_{wn}",
                                  name=f"w_{m}_{wn}")
                    if gated:
                        gated_dma(t, dram[f"{m}_{wn}"][:])
                    else:
                        half = NT // 2
                        nc.sync.dma_start(t[:, 0:half],
                                          dram[f"{m}_{wn}"][:, 0:half])
                        nc.sync.dma_start(t[:, half:NT],
                                          dram[f"{m}_{wn}"][:, half:NT])
                    ts[wn] = t
                return ts

            # ---------------- projection emitters (generators) ----------
            def v_proj_chunks(wv, V):
                """Yield after each s-tile: V[st] = (x @ vw) * DESC, bf16."""
                for st in range(KT):
                    ssl = slice(st * P, (st + 1) * P)
                    pst = [pproj.tile([P, 512], F32, tag="pj",
                                      name=f"pjv_{id(V)}_{st}_{i}")
                           for i in range(2)]
                    for pr in range(NPAIR):
                        ksl = slice(2 * pr, 2 * pr + 2)
                        for hc in range(2):
                            hsl = slice(hc * 512, (hc + 1) * 512)
                            nc.tensor.matmul(
                                pst[hc][:], lhsT=xT[:, ksl, ssl],
                                rhs=wv[:, ksl, hsl],
                                start=(pr == 0), stop=(pr == NPAIR - 1),
                                perf_mode=DR)
                    for hc in range(2):
                        hsl = slice(hc * 512, (hc + 1) * 512)
                        nc.vector.tensor_scalar_mul(V[:, st, hsl],
                                                    pst[hc][:], DESC / SWP)
                    yield

            def qk_proj_chunks(wq, wk, qb):
                """Yield after each (weight, ho) unit. Q covers Sq query
                positions; K covers all Sp key positions."""
                for wn, wt, dst, tot in (("q", wq, QT8, Sq),
                                         ("k", wk, KT8, Sp)):
                    for ho in range(NT):
                        hsl = slice(ho * P, (ho + 1) * P)
                        cks = _chunks(tot)
                        pst = [pproj.tile([P, 512], F32, tag="pj",
                                          name=f"pj{wn}_{id(wt)}_{ho}_{i}")
                               for i in range(len(cks))]
                        for pr in range(NPAIR):
                            ksl = slice(2 * pr, 2 * pr + 2)
                            for ci, (o, w) in enumerate(cks):
                                nc.tensor.matmul(
                                    pst[ci][:, 0:w], lhsT=wt[:, ksl, hsl],
                                    rhs=xT[:, ksl, o:o + w],
                                    start=(pr == 0), stop=(pr == NPAIR - 1),
                                    perf_mode=DR)
                        for ci, (o, w) in enumerate(cks):
                            if wn == "q":
                                nc.vector.tensor_scalar(
                                    dst[:, ho, o:o + w], pst[ci][:, 0:w],
                                    scalar1=DESC, scalar2=qb[:, ho:ho + 1],
                                    op0=ALU.mult, op1=ALU.add)
                            else:
                                nc.vector.tensor_scalar_mul(
                                    dst[:, ho, o:o + w], pst[ci][:, 0:w],
                                    DESC)
                        yield

            def drain(gen, n=1):
                if gen is None:
                    return
                for _ in range(n):
                    try:
                        next(gen)
                    except StopIteration:
                        break

            def drain_all(gen):
                if gen is None:
                    return
                for _ in gen:
                    pass

            # ---------------- first branch Q/K projection ----------------
            m0, _ = MHAS[0]
            w_cur = {}
            for wn in ("qw", "kw"):
                t = wres.tile([P, NT, H], FP8, tag=f"w_{wn}",
                              name=f"w_{m0}_{wn}")
                half = NT // 2
                nc.sync.dma_start(t[:, 0:half], dram[f"{m0}_{wn}"][:, 0:half])
                nc.sync.dma_start(t[:, half:NT], dram[f"{m0}_{wn}"][:, half:NT])
                w_cur[wn] = t
            qb_t = biasp.tile([P, NT], F32, tag="qb", name="qb0")
            nc.sync.dma_start(qb_t[:], dram[f"{m0}_qb"][:])
            vb_t = biasp.tile([P, NT], F32, tag="vb", name="vb0")
            nc.sync.dma_start(vb_t[:], dram[f"{m0}_vb"][:])
            # PE warm-up while the prologue DMAs land: keeps the HAM
            # clock-gate at full rate so the first real matmuls run warm
            wrm = small.tile([P, 512], FP8, tag="wrm")
            nc.vector.memset(wrm[:], 0.0)
            _warm_ct = [0]

            def warm(n=6):
                _warm_ct[0] += 1
                pw_t = pproj.tile([P, 512], F32, tag="pj",
                                  name=f"warm{_warm_ct[0]}")
                for i in range(n):
                    nc.tensor.matmul(pw_t[:], lhsT=wrm[:, 0:128],
                                     rhs=wrm[:], start=(i == 0),
                                     stop=(i == n - 1))

            warm(48)
            qk0 = qk_proj_chunks(w_cur["qw"], w_cur["kw"], qb_t)
            drain(qk0, 10)
            t = wres.tile([P, NT, H], FP8, tag="w_vw", name=f"w_{m0}_vw")
            gated_dma(t, dram[f"{m0}_vw"][:])
            w_cur["vw"] = t
            drain_all(qk0)

            oww1 = wres.tile([P, NT, H], BF16, tag="w_oww1", name="oww1_0")
            gated_dma(oww1, dram[f"{m0}_oww1"][:])
            f2r = small.tile([P, NT, H // 2], BF16, tag="f2r")
            gated_dma(f2r, dram["fus2_w"][:])
            clsr = small.tile([P, 4, 2], BF16, tag="clsr")
            nc.sync.dma_start(clsr[:], dram["cls_w"][:])
            b2r4 = small.tile([1, 4, P], BF16, tag="b2r4")
            nc.sync.dma_start(b2r4[:], dram["b2r4"][:])
            oneb = small.tile([1, 1], BF16, tag="oneb")
            nc.vector.memset(oneb[:], 1.0)
            cbrow = small.tile([1, 2], F32, tag="cbrow")
            nc.sync.dma_start(cbrow[:], dram["cbrow"][:])

            kcks = _chunks(Sp, 512)      # scores psum chunks: matmul
                                         # writes must not cross banks

            tail_gen = None              # previous branch's epilogue units

            for mi, (m, nh) in enumerate(MHAS):
                d = H // nh
                ndt = d // P
                inv_sqrt_d = 1.0 / float(np.sqrt(d))

                # this branch's V projection (drained inside the sweep)
                V_cur = vbuf.tile([P, KT, H], BF16, tag="V",
                                  name=f"V_{m}")
                v_gen = v_proj_chunks(w_cur["vw"], V_cur)

                # prefetch next branch weights + Q/K emitter
                qk_gen = None
                if mi + 1 < len(MHAS):
                    mn, _ = MHAS[mi + 1]
                    w_nxt = load_branch_weights(mn, gated=True)
                    qb_n = biasp.tile([P, NT], F32, tag="qb",
                                      name=f"qb{mi + 1}")
                    nc.sync.dma_start(qb_n[:], dram[f"{mn}_qb"][:])
                    vb_n = biasp.tile([P, NT], F32, tag="vb",
                                      name=f"vb{mi + 1}")
                    nc.sync.dma_start(vb_n[:], dram[f"{mn}_vb"][:])
                    qk_gen = qk_proj_chunks(w_nxt["qw"], w_nxt["kw"], qb_n)

                EXP = expp.tile([P, QT, 8, Sp], FP8, tag="expb",
                                name=f"EXP{mi}")
                den = work.tile([P, QT, 8], F32, tag="den",
                                name=f"den{mi}")
                nc.vector.memset(den[:], 1.0)
                wp8 = work.tile([P, QT, 8], FP8, tag="wp8",
                                name=f"wp8{mi}")
                GC = KT * 8
                pg = pgp.tile([P, GC + 16], F32, tag="g", name=f"pg{mi}")
                G16 = work.tile([P, KT, 8], BF16, tag="G16",
                                name=f"G16{mi}")

                # front-loaded V drain so pooled can pipeline per head
                vplan = _spread(KT, min(nh, 2))
                vplan += [0] * (nh - len(vplan))
                qkplan = _spread(2 * NT, nh)

                def head_rec(h):
                    rec = work.tile([P, QT], F32, tag="rec",
                                    name=f"rec{mi}_{h}")
                    nc.vector.tensor_scalar_add(
                        rec[:], den[:, :, h], npadneg[:, 0:1])
                    nc.vector.reciprocal(rec[:], rec[:])
                    nc.vector.tensor_tensor(
                        out=wp8[:, :, h], in0=rec[:], in1=pw[:],
                        op=ALU.mult)

                def head_g(h):
                    for kt in range(KT):
                        for qt in range(QT):
                            qn = P if qt < QT - 1 else QL
                            nc.tensor.matmul(
                                pg[:, kt * 8 + h:kt * 8 + h + 1],
                                lhsT=EXP[0:qn, qt, h,
                                         kt * P:(kt + 1) * P],
                                rhs=wp8[0:qn, qt, h:h + 1],
                                start=(qt == 0), stop=(qt == QT - 1))

                def head_pooled(h, _pg=pg, _G16=G16, _V=V_cur,
                                _ndt=ndt, _GC=GC):
                    """G16 slice + pooled columns owned by head h."""
                    nc.vector.tensor_copy(
                        _G16[:, :, h],
                        _pg[:, 0:_GC].rearrange("p (k h) -> p k h",
                                                k=KT)[:, :, h])
                    for dc in range(h * _ndt, (h + 1) * _ndt):
                        for kt in range(KT):
                            nc.tensor.matmul(
                                _pg[:, _GC + dc:_GC + dc + 1],
                                lhsT=_V[:, kt, dc * P:(dc + 1) * P],
                                rhs=_G16[:, kt, h:h + 1],
                                start=(kt == 0), stop=(kt == KT - 1))

                def branch_tail(vb_cur, oww1_cur, pooled_left, _mi=mi,
                                _pg=pg, _GC=GC, _hp=head_pooled):
                    """Epilogue units; yields between PE clumps."""
                    for h in pooled_left:
                        _hp(h)
                    yield
                    pb = work.tile([P, NT], BF16, tag="poolb",
                                   name=f"pb{_mi}")
                    nc.vector.tensor_tensor(out=pb[:], in0=_pg[:, _GC:_GC + 8],
                                            in1=vb_cur[:], op=ALU.add)
                    for oc in range(NT):
                        for kt in range(NT):
                            nc.tensor.matmul(
                                _pg[:, _GC + 8 + oc:_GC + 9 + oc],
                                lhsT=oww1_cur[:, kt, oc * P:(oc + 1) * P],
                                rhs=pb[:, kt:kt + 1],
                                start=(kt == 0), stop=(kt == NT - 1))
                    yield
                    nc.vector.tensor_add(out=h1acc[:], in0=h1acc[:],
                                         in1=_pg[:, _GC + 8:_GC + 16])
                    yield

                # ---- per-head pipelined sweep ---------------------------
                # aux queue: previous branch's epilogue, then next
                # branch's Q/K projection, drained across fixed slots
                import itertools
                aux_srcs = [g for g in (tail_gen, qk_gen) if g is not None]
                aux = itertools.chain(*aux_srcs) if aux_srcs else None
                n_units = (3 if tail_gen is not None else 0) + \
                          (2 * NT if qk_gen is not None else 0)
                slots = []
                for h in range(nh):
                    for qt in range(QT):
                        if qt == 1 and h == 0:
                            slots.append((h, qt))
                        elif qt not in (0, 1) and not (qt == 3 and h >= 2):
                            slots.append((h, qt))
                auxplan = {hq: n for hq, n in
                           zip(slots, _spread(n_units, max(1, len(slots))))}
                for h in range(nh):
                    for qt in range(QT):
                        qn = P if qt < QT - 1 else QL
                        qsl = slice(qt * P, qt * P + qn)
                        sc = psc.tile([P, Sp], F32, tag="sc",
                                      name=f"sc{mi}_{h}_{qt}")
                        for (o, w) in kcks:
                            if ndt == 2:
                                nc.tensor.matmul(
                                    sc[0:qn, o:o + w],
                                    lhsT=QT8[:, 2 * h:2 * h + 2, qsl],
                                    rhs=KT8[:, 2 * h:2 * h + 2, o:o + w],
                                    start=True, stop=True, perf_mode=DR)
                            else:
                                nc.tensor.matmul(
                                    sc[0:qn, o:o + w],
                                    lhsT=QT8[:, h, qsl],
                                    rhs=KT8[:, h, o:o + w],
                                    start=True, stop=True)
                        nc.scalar.activation(
                            EXP[0:qn, qt, h, :], sc[0:qn, :], AF.Exp,
                            scale=inv_sqrt_d,
                            accum_out=den[0:qn, qt, h:h + 1])
                        # interleave aux PE work between score tiles
                        if qt == 0:
                            if h > 0:
                                head_rec(h - 1)
                            drain(v_gen, vplan[h])
                        elif qt == 1 and h > 0:
                            head_g(h - 1)
                        elif qt == 3 and h >= 2:
                            head_pooled(h - 2)
                        else:
                            drain(aux, auxplan.get((h, qt), 0))
                head_rec(nh - 1)
                head_g(nh - 1)
                if mi + 1 == len(MHAS) and nh >= 2:
                    # deps (g, V) are ready; shortens the cold serial tail
                    head_pooled(nh - 2)
                    pooled_left = [nh - 1]
                else:
                    pooled_left = [nh - 2, nh - 1] if nh >= 2 else [0]
                drain_all(v_gen)
                drain_all(aux)

                tail_gen = branch_tail(vb_t, oww1, pooled_left)
                if mi + 1 == len(MHAS):
                    drain(tail_gen, 1)
                    warm()
                    drain(tail_gen, 1)
                    warm()
                    drain_all(tail_gen)
                    tail_gen = None

                # rotate per-branch state
                if mi + 1 < len(MHAS):
                    w_cur = w_nxt
                    qb_t, vb_t = qb_n, vb_n
                    mn, _ = MHAS[mi + 1]
                    oww1 = wres.tile([P, NT, H], BF16, tag="w_oww1",
                                     name=f"oww1_{mn}")
                    gated_dma(oww1, dram[f"{mn}_oww1"][:])

            # ---------------- MLP tail (all column-form) ------------------
            h1rc = small.tile([P, NT], BF16, tag="h1rc")
            nc.vector.tensor_relu(h1rc[:], h1acc[:])
            warm()
            tpg = pgp.tile([P, 4], F32, tag="g", name="tailpg")
            for oc in range(4):
                nc.tensor.matmul(tpg[:, oc:oc + 1],
                                 lhsT=b2r4[0:1, oc, :], rhs=oneb[:],
                                 start=True, stop=False)
                for kt in range(NT):
                    nc.tensor.matmul(
                        tpg[:, oc:oc + 1],
                        lhsT=f2r[:, kt, oc * P:(oc + 1) * P],
                        rhs=h1rc[:, kt:kt + 1],
                        start=False, stop=(kt == NT - 1))
            h2rc = small.tile([P, 4], BF16, tag="h2rc")
            nc.vector.tensor_relu(h2rc[:], tpg[:, 0:4])
            warm()

            plg = psc.tile([1, 2], F32, tag="sc", name="lg")
            for kt in range(4):
                nc.tensor.matmul(plg[:], lhsT=h2rc[:, kt:kt + 1],
                                 rhs=clsr[:, kt], start=(kt == 0),
                                 stop=(kt == 3))
            lg = small.tile([1, 2], F32, tag="lgsb")
            nc.vector.tensor_add(out=lg[:], in0=plg[:], in1=cbrow[:])
            nc.sync.dma_start(out[:], lg[:])

    _split_multi_waits(nc)
    return nc


def _split_multi_waits(nc, max_on_inst=1, max_on_evsem=2):
    """This walrus build caps sync waits per instruction at 1 (2 for
    EventSemaphore); Tile attaches one wait per dependent proc. Spill excess
    waits onto pure-wait EventSemaphores inserted before, on the same engine -
    the engine blocks on each condition in sequence, so semantics match."""
    for f in nc.m.functions:
        for bb in f.blocks:
            insts = list(bb.instructions)
            new = []
            changed = False
            for ins in insts:
                si = ins.sync_info
                if si is not None:
                    waits = list(si.on_wait)
                    cap = (max_on_evsem
                           if isinstance(ins, mybir.InstEventSemaphore)
                           else max_on_inst)
                    if len(waits) > cap:
                        spill = waits[:-cap]
                        keep = waits[-cap:]
                        k = 0
                        while spill:
                            chunk = spill[:max_on_evsem]
                            spill = spill[max_on_evsem:]
                            new.append(mybir.InstEventSemaphore(
                                name=f"{ins.name}-wspill{k}",
                                engine=ins.engine, ins=[], outs=[],
                                sync_info=mybir.SyncInfo(on_wait=chunk,
                                                         on_update=[])))
                            k += 1
                        ins.sync_info = mybir.SyncInfo(
                            on_wait=keep, on_update=list(si.on_update))
                        changed = True
                new.append(ins)
            if changed:
                bb.instructions = new


def _get_nc(Sp, Sq):
    if (Sp, Sq) not in _CACHE:
        _CACHE[(Sp, Sq)] = _build_nc(Sp, Sq)
    return _CACHE[(Sp, Sq)]


def _q8(a, scale):
    return np.clip(a.astype(np.float32) * scale, -240.0, 240.0).astype(E4)


def _h3(a):
    """[K, N] -> [P, K//P, N] partition-inner, contiguous."""
    K, N = a.shape
    return np.ascontiguousarray(a.reshape(K // P, P, N).transpose(1, 0, 2))


def _prep_in_maps(inputs, Sp, Sq):
    f32 = np.float32
    QT = -(-Sq // P)
    mask = inputs["attention_mask"].astype(np.int64)     # [B, S]

    w1 = inputs["fus1_w"].astype(f32)                    # [3H, H]
    shared = {
        "b2r4": np.ascontiguousarray(
            inputs["fus2_b"].astype(BF).reshape(1, 4, P)),
        "cbrow": inputs["cls_b"].astype(f32).reshape(1, 2),
        "fus2_w": _h3(inputs["fus2_w"].astype(BF)),
        "cls_w": _h3(inputs["cls_w"].astype(BF)),
    }

    b1 = inputs["fus1_b"].astype(f32).copy()
    for mi, (m, _) in enumerate(MHAS):
        w1m = w1[mi * H:(mi + 1) * H]                    # [H, H]
        for wn in ("qw", "kw", "vw"):
            shared[f"{m}_{wn}"] = _h3(_q8(inputs[f"{m}_{wn}"], SW))
        oww1 = inputs[f"{m}_ow"].astype(f32) @ w1m
        shared[f"{m}_oww1"] = _h3(oww1.astype(BF))
        b1 += inputs[f"{m}_ob"].astype(f32) @ w1m
        shared[f"{m}_qb"] = np.ascontiguousarray(
            inputs[f"{m}_qb"].astype(f32).reshape(NT, P).T)
        shared[f"{m}_vb"] = np.ascontiguousarray(
            inputs[f"{m}_vb"].astype(f32).reshape(NT, P).T)
    shared["b1col"] = np.ascontiguousarray(b1.reshape(NT, P).T)

    in_maps = []
    for c in range(NCORES):
        im = dict(shared)
        idx = np.nonzero(mask[c])[0]
        nv = len(idx)
        xp = np.zeros((Sp, H), f32)
        xp[:nv] = inputs["hidden_states"][c][idx]
        im["xT"] = _h3(_q8(xp.T, SX))
        pwv = np.zeros(QT * P, f32)
        pwv[:nv] = SWP / nv
        im["pw"] = np.ascontiguousarray(
            pwv.reshape(QT, P).T.astype(f32))
        im["npadneg"] = np.full((P, 1), -(Sp - nv), f32)
        in_maps.append(im)
    return in_maps


def kernel(**inputs) -> np.ndarray:
    mask = inputs["attention_mask"]
    maxc = int(mask.astype(np.int64).sum(axis=1).max())
    Sp = min(S, max(P, -(-maxc // P) * P))
    Sq = min(Sp, max(64, -(-maxc // 64) * 64))
    nc = _get_nc(Sp, Sq)
    in_maps = _prep_in_maps(inputs, Sp, Sq)
    res = run_bass_kernel_spmd(nc, in_maps, core_ids=list(range(NCORES)))
    return np.concatenate(
        [res.results[c]["out"] for c in range(NCORES)], axis=0
    ).astype(np.float32)


# revision 29
# speedup vs baseline: 1.0490x; 1.0453x over previous
"""EngagementPredictor TRN2 kernel: 3-branch MHA + masked mean-pool + MLP.

Sharding: pure data-parallel - B=8 batch elements, one per NeuronCore;
weights replicated; no collectives. Each core computes its [2]-logit row.

v3 design (over the bf16 baseline):
  * Valid-token packing as before (Sp keys / Sq queries, zero-padded).
  * fp8(e4m3) DoubleRow matmuls for all Q/K/V projections: weights are
    host-quantized at x4096, x at x32, contraction pairs two 128-row
    k-tiles per instruction -> half the PE stream cycles.
  * Scores in [query-part, key-free] orientation (lhsT=Q). The exp
    activation produces the softmax denominator for free via accum_out
    (free-axis sum); padded keys have x=0 -> score 0 -> exp=1, their
    count is subtracted from the denominator and their V rows are
    exactly zero, so no mask tensor is needed at all.
  * Pooling: pooled[d] = sum_k V[k,d] * g[k], g[k] = sum_q exp[q,k] *
    (poolw[q]/den[q]); g and pooled are N=1 matmuls (no ctx
    materialization, no broadcast, no big DVE reductions).
  * tmp/pat scores (d=256) run DoubleRow over their two d-tiles.
  * Issue-order software pipelining tuned for the in-order engine
    queues: each branch drains its OWN V projection plus the next
    branch's Q/K projection inside the scores sweep, interleaved
    between score matmuls so the PE never head-of-line blocks on the
    scalar exp chain; g matmuls of head h-1 ride between the scores
    of head h.
  * fus1 and the MLP tail are kept in column form end-to-end: the
    o-proj+fus1 GEMV accumulates [128, 8] per branch via N=1 matmuls,
    so no row->column transposes and a short serial tail.
"""
import numpy as np
import ml_dtypes

import concourse.bass as bass
import concourse.tile as tile
from concourse import mybir
from concourse.bass_utils import run_bass_kernel_spmd

F32 = mybir.dt.float32
BF16 = mybir.dt.bfloat16
FP8 = mybir.dt.float8e4
AF = mybir.ActivationFunctionType
ALU = mybir.AluOpType
DR = mybir.MatmulPerfMode.DoubleRow
BF = ml_dtypes.bfloat16
E4 = ml_dtypes.float8_e4m3fn

P = 128
S = 1024
H = 1024
NT = H // P          # 8 tiles of 128 along H
NCORES = 8
MHAS = [("beh", 8), ("tmp", 4), ("pat", 4)]

SX = 32.0            # x fp8 scale
SW = 4096.0          # weight fp8 scale
DESC = 1.0 / (SX * SW)
SWP = float(2 ** 19)  # w' fp8 scale (undone after pooling)

_CACHE = {}


def _chunks(total, step=None):
    if step is None:
        step = -(-total // -(-total // 512))   # equal-ish chunks <= 512
    out = []
    o = 0
    while o < total:
        c = min(step, total - o)
        out.append((o, c))
        o += c
    return out


def _spread(n_units, n_heads):
    """Distribute n_units drain units over n_heads heads, front-loaded."""
    base = n_units // n_heads
    extra = n_units - base * n_heads
    return [base + (1 if h < extra else 0) for h in range(n_heads)]


def _build_nc(Sp, Sq):
    KT = Sp // P                 # key tiles
    QT = -(-Sq // P)             # query tiles (last may be partial)
    QL = Sq - (QT - 1) * P       # rows in last query tile
    NPAIR = NT // 2

    nc = bass.Bass()
    dram = {}

    def dp(name, shape, dt=FP8):
        dram[name] = nc.declare_dram_parameter(name, list(shape), dt,
                                               isOutput=False)

    dp("xT", (P, NT, Sp))
    dp("pw", (P, QT), F32)        # SWP/nv on valid queries, else 0
    dp("npadneg", (P, 1), F32)    # -(Sp - nv), replicated
    for m, _ in MHAS:
        for wn in ("qw", "kw", "vw"):
            dp(f"{m}_{wn}", (P, NT, H))
        dp(f"{m}_oww1", (P, NT, H), BF16)
        dp(f"{m}_qb", (P, NT), F32)
        dp(f"{m}_vb", (P, NT), F32)
    dp("b1col", (P, NT), F32)     # fus1_b + sum_m ob_m @ fus1_w[m]
    dp("fus2_w", (P, NT, H // 2), BF16)
    dp("b2r4", (1, 4, P), BF16)
    dp("cls_w", (P, 4, 2), BF16)
    dp("cbrow", (1, 2), F32)
    out = nc.declare_dram_parameter("out", [1, 2], F32, isOutput=True)

    with tile.TileContext(nc) as tc, \
         nc.allow_low_precision(
             reason="fp8 matmul operands with fp32 PSUM accumulation; "
                    "bias/pool math in fp32 on DVE"):
        with tc.tile_pool(name="big", bufs=1) as big, \
             tc.tile_pool(name="vbuf", bufs=2) as vbuf, \
             tc.tile_pool(name="wres", bufs=2) as wres, \
             tc.tile_pool(name="expp", bufs=2) as expp, \
             tc.tile_pool(name="small", bufs=1) as small, \
             tc.tile_pool(name="bias", bufs=3) as biasp, \
             tc.tile_pool(name="work", bufs=2) as work, \
             tc.tile_pool(name="pproj", bufs=3, space="PSUM") as pproj, \
             tc.tile_pool(name="psc", bufs=2, space="PSUM") as psc, \
             tc.tile_pool(name="pg", bufs=1, space="PSUM") as pgp:

            # ---- resident inputs ----
            xT = big.tile([P, NT, Sp], FP8, tag="xT")
            nc.sync.dma_start(xT[:], dram["xT"][:])
            QT8 = big.tile([P, NT, Sq], FP8, tag="QT8")
            KT8 = big.tile([P, NT, Sp], FP8, tag="KT8")

            pw = small.tile([P, QT], F32, tag="pw")
            nc.sync.dma_start(pw[:], dram["pw"][:])
            npadneg = small.tile([P, 1], F32, tag="npadneg")
            nc.sync.dma_start(npadneg[:], dram["npadneg"][:])

            h1acc = small.tile([P, NT], F32, tag="h1acc")
            nc.sync.dma_start(h1acc[:], dram["b1col"][:])

            def gated_dma(t, src):
                """DMA whose trigger waits for the DVE stream to reach
                this program point (WAW dep on a tiny memset), so bulk
                prefetches don't steal HBM bandwidth from earlier
                critical loads."""
                nc.vector.memset(t[0:1, 0:1, 0:1], 0.0)
                nc.sync.dma_start(t[:], src)

            def load_branch_weights(m, gated=False):
                ts = {}
                for wn in ("qw", "kw", "vw"):
                    t = wres.tile([P, NT, H], FP8, tag=f"w_{wn}",
                                  name=f"w_{m}_{wn}")
                    if gated:
                        gated_dma(t, dram[f"{m}_{wn}"][:])
                    else:
                        half = NT // 2
                        nc.sync.dma_start(t[:, 0:half],
                                          dram[f"{m}_{wn}"][:, 0:half])
                        nc.sync.dma_start(t[:, half:NT],
                                          dram[f"{m}_{wn}"][:, half:NT])
                    ts[wn] = t
                return ts

            # ---------------- projection emitters (generators) ----------
            def v_proj_chunks(wv, V):
                """Yield after each s-tile: V[st] = (x @ vw) * DESC, bf16."""
                for st in range(KT):
                    ssl = slice(st * P, (st + 1) * P)
                    pst = [pproj.tile([P, 512], F32, tag="pj",
                                      name=f"pjv_{id(V)}_{st}_{i}")
                           for i in range(2)]
                    for pr in range(NPAIR):
                        ksl = slice(2 * pr, 2 * pr + 2)
                        for hc in range(2):
                            hsl = slice(hc * 512, (hc + 1) * 512)
                            nc.tensor.matmul(
                                pst[hc][:], lhsT=xT[:, ksl, ssl],
                                rhs=wv[:, ksl, hsl],
                                start=(pr == 0), stop=(pr == NPAIR - 1),
                                perf_mode=DR)
                    for hc in range(2):
                        hsl = slice(hc * 512, (hc + 1) * 512)
                        nc.vector.tensor_scalar_mul(V[:, st, hsl],
                                                    pst[hc][:], DESC / SWP)
                    yield

            def qk_proj_chunks(wq, wk, qb):
                """Yield after each (weight, ho) unit. Q covers Sq query
                positions; K covers all Sp key positions."""
                for wn, wt, dst, tot in (("q", wq, QT8, Sq),
                                         ("k", wk, KT8, Sp)):
                    for ho in range(NT):
                        hsl = slice(ho * P, (ho + 1) * P)
                        cks = _chunks(tot)
                        pst = [pproj.tile([P, 512], F32, tag="pj",
                                          name=f"pj{wn}_{id(wt)}_{ho}_{i}")
                               for i in range(len(cks))]
                        for pr in range(NPAIR):
                            ksl = slice(2 * pr, 2 * pr + 2)
                            for ci, (o, w) in enumerate(cks):
                                nc.tensor.matmul(
                                    pst[ci][:, 0:w], lhsT=wt[:, ksl, hsl],
                                    rhs=xT[:, ksl, o:o + w],
                                    start=(pr == 0), stop=(pr == NPAIR - 1),
                                    perf_mode=DR)
                        for ci, (o, w) in enumerate(cks):
                            if wn == "q":
                                nc.vector.tensor_scalar(
                                    dst[:, ho, o:o + w], pst[ci][:, 0:w],
                                    scalar1=DESC, scalar2=qb[:, ho:ho + 1],
                                    op0=ALU.mult, op1=ALU.add)
                            else:
                                nc.vector.tensor_scalar_mul(
                                    dst[:, ho, o:o + w], pst[ci][:, 0:w],
                                    DESC)
                        yield

            def drain(gen, n=1):
                if gen is None:
                    return
                for _ in range(n):
                    try:
                        next(gen)
                    except StopIteration:
                        break

            def drain_all(gen):
                if gen is None:
                    return
                for _ in gen:
                    pass

            # ---------------- first branch Q/K projection ----------------
            m0, _ = MHAS[0]
            w_cur = {}
            for wn in ("qw", "kw"):
                t = wres.tile([P, NT, H], FP8, tag=f"w_{wn}",
                              name=f"w_{m0}_{wn}")
                half = NT // 2
                nc.sync.dma_start(t[:, 0:half], dram[f"{m0}_{wn}"][:, 0:half])
                nc.sync.dma_start(t[:, half:NT], dram[f"{m0}_{wn}"][:, half:NT])
                w_cur[wn] = t
            qb_t = biasp.tile([P, NT], F32, tag="qb", name="qb0")
            nc.sync.dma_start(qb_t[:], dram[f"{m0}_qb"][:])
            vb_t = biasp.tile([P, NT], F32, tag="vb", name="vb0")
            nc.sync.dma_start(vb_t[:], dram[f"{m0}_vb"][:])
            # PE warm-up while the prologue DMAs land: keeps the HAM
            # clock-gate at full rate so the first real matmuls run warm
            wrm = small.tile([P, 512], FP8, tag="wrm")
            nc.vector.memset(wrm[:], 0.0)
            _warm_ct = [0]

            def warm(n=6):
                _warm_ct[0] += 1
                pw_t = pproj.tile([P, 512], F32, tag="pj",
                                  name=f"warm{_warm_ct[0]}")
                for i in range(n):
                    nc.tensor.matmul(pw_t[:], lhsT=wrm[:, 0:128],
                                     rhs=wrm[:], start=(i == 0),
                                     stop=(i == n - 1))

            warm(48)
            qk0 = qk_proj_chunks(w_cur["qw"], w_cur["kw"], qb_t)
            drain(qk0, 10)
            t = wres.tile([P, NT, H], FP8, tag="w_vw", name=f"w_{m0}_vw")
            gated_dma(t, dram[f"{m0}_vw"][:])
            w_cur["vw"] = t
            drain_all(qk0)

            oww1 = wres.tile([P, NT, H], BF16, tag="w_oww1", name="oww1_0")
            gated_dma(oww1, dram[f"{m0}_oww1"][:])
            f2r = small.tile([P, NT, H // 2], BF16, tag="f2r")
            gated_dma(f2r, dram["fus2_w"][:])
            clsr = small.tile([P, 4, 2], BF16, tag="clsr")
            nc.sync.dma_start(clsr[:], dram["cls_w"][:])
            b2r4 = small.tile([1, 4, P], BF16, tag="b2r4")
            nc.sync.dma_start(b2r4[:], dram["b2r4"][:])
            oneb = small.tile([1, 1], BF16, tag="oneb")
            nc.vector.memset(oneb[:], 1.0)
            cbrow = small.tile([1, 2], F32, tag="cbrow")
            nc.sync.dma_start(cbrow[:], dram["cbrow"][:])

            kcks = _chunks(Sp, 512)      # scores psum chunks: matmul
                                         # writes must not cross banks

            tail_gen = None              # previous branch's epilogue units

            for mi, (m, nh) in enumerate(MHAS):
                d = H // nh
                ndt = d // P
                inv_sqrt_d = 1.0 / float(np.sqrt(d))

                # this branch's V projection (drained inside the sweep)
                V_cur = vbuf.tile([P, KT, H], BF16, tag="V",
                                  name=f"V_{m}")
                v_gen = v_proj_chunks(w_cur["vw"], V_cur)

                # prefetch next branch weights + Q/K emitter
                qk_gen = None
                if mi + 1 < len(MHAS):
                    mn, _ = MHAS[mi + 1]
                    w_nxt = load_branch_weights(mn, gated=True)
                    qb_n = biasp.tile([P, NT], F32, tag="qb",
                                      name=f"qb{mi + 1}")
                    nc.sync.dma_start(qb_n[:], dram[f"{mn}_qb"][:])
                    vb_n = biasp.tile([P, NT], F32, tag="vb",
                                      name=f"vb{mi + 1}")
                    nc.sync.dma_start(vb_n[:], dram[f"{mn}_vb"][:])
                    qk_gen = qk_proj_chunks(w_nxt["qw"], w_nxt["kw"], qb_n)

                EXP = expp.tile([P, QT, 8, Sp], FP8, tag="expb",
                                name=f"EXP{mi}")
                den = work.tile([P, QT, 8], F32, tag="den",
                                name=f"den{mi}")
                nc.vector.memset(den[:], 1.0)
                wp8 = work.tile([P, QT, 8], FP8, tag="wp8",
                                name=f"wp8{mi}")
                GC = KT * 8
                pg = pgp.tile([P, GC + 16], F32, tag="g", name=f"pg{mi}")
                G16 = work.tile([P, KT, 8], BF16, tag="G16",
                                name=f"G16{mi}")

                # front-loaded V drain so pooled can pipeline per head
                vplan = _spread(KT, min(nh, 2))
                vplan += [0] * (nh - len(vplan))
                qkplan = _spread(2 * NT, nh)

                def head_rec(h):
                    rec = work.tile([P, QT], F32, tag="rec",
                                    name=f"rec{mi}_{h}")
                    nc.vector.tensor_scalar_add(
                        rec[:], den[:, :, h], npadneg[:, 0:1])
                    nc.vector.reciprocal(rec[:], rec[:])
                    nc.vector.tensor_tensor(
                        out=wp8[:, :, h], in0=rec[:], in1=pw[:],
                        op=ALU.mult)

                def head_g(h):
                    for kt in range(KT):
                        for qt in range(QT):
                            qn = P if qt < QT - 1 else QL
                            nc.tensor.matmul(
                                pg[:, kt * 8 + h:kt * 8 + h + 1],
                                lhsT=EXP[0:qn, qt, h,
                                         kt * P:(kt + 1) * P],
                                rhs=wp8[0:qn, qt, h:h + 1],
                                start=(qt == 0), stop=(qt == QT - 1))

                def head_pooled(h, _pg=pg, _G16=G16, _V=V_cur,
                                _ndt=ndt, _GC=GC):
                    """G16 slice + pooled columns owned by head h."""
                    nc.vector.tensor_copy(
                        _G16[:, :, h],
                        _pg[:, 0:_GC].rearrange("p (k h) -> p k h",
                                                k=KT)[:, :, h])
                    for dc in range(h * _ndt, (h + 1) * _ndt):
                        for kt in range(KT):
                            nc.tensor.matmul(
                                _pg[:, _GC + dc:_GC + dc + 1],
                                lhsT=_V[:, kt, dc * P:(dc + 1) * P],
                                rhs=_G16[:, kt, h:h + 1],
                                start=(kt == 0), stop=(kt == KT - 1))

                def branch_tail(vb_cur, oww1_cur, pooled_left, _mi=mi,
                                _pg=pg, _GC=GC, _hp=head_pooled):
                    """Epilogue units; yields between PE clumps."""
                    for h in pooled_left:
                        _hp(h)
                    yield
                    pb = work.tile([P, NT], BF16, tag="poolb",
                                   name=f"pb{_mi}")
                    nc.vector.tensor_tensor(out=pb[:], in0=_pg[:, _GC:_GC + 8],
                                            in1=vb_cur[:], op=ALU.add)
                    for oc in range(NT):
                        for kt in range(NT):
                            nc.tensor.matmul(
                                _pg[:, _GC + 8 + oc:_GC + 9 + oc],
                                lhsT=oww1_cur[:, kt, oc * P:(oc + 1) * P],
                                rhs=pb[:, kt:kt + 1],
                                start=(kt == 0), stop=(kt == NT - 1))
                    yield
                    nc.vector.tensor_add(out=h1acc[:], in0=h1acc[:],
                                         in1=_pg[:, _GC + 8:_GC + 16])
                    yield

                # ---- per-head pipelined sweep ---------------------------
                # aux queue: previous branch's epilogue, then next
                # branch's Q/K projection, drained across fixed slots
                import itertools
                aux_srcs = [g for g in (tail_gen, qk_gen) if g is not None]
                aux = itertools.chain(*aux_srcs) if aux_srcs else None
                n_units = (3 if tail_gen is not None else 0) + \
                          (2 * NT if qk_gen is not None else 0)
                slots = []
                for h in range(nh):
                    for qt in range(QT):
                        if qt == 1 and h == 0:
                            slots.append((h, qt))
                        elif qt not in (0, 1) and not (qt == 3 and h >= 2):
                            slots.append((h, qt))
                auxplan = {hq: n for hq, n in
                           zip(slots, _spread(n_units, max(1, len(slots))))}
                for h in range(nh):
                    for qt in range(QT):
                        qn = P if qt < QT - 1 else QL
                        qsl = slice(qt * P, qt * P + qn)
                        sc = psc.tile([P, Sp], F32, tag="sc",
                                      name=f"sc{mi}_{h}_{qt}")
                        for (o, w) in kcks:
                            if ndt == 2:
                                nc.tensor.matmul(
                                    sc[0:qn, o:o + w],
                                    lhsT=QT8[:, 2 * h:2 * h + 2, qsl],
                                    rhs=KT8[:, 2 * h:2 * h + 2, o:o + w],
                                    start=True, stop=True, perf_mode=DR)
                            else:
                                nc.tensor.matmul(
                                    sc[0:qn, o:o + w],
                                    lhsT=QT8[:, h, qsl],
                                    rhs=KT8[:, h, o:o + w],
                                    start=True, stop=True)
                        nc.scalar.activation(
                            EXP[0:qn, qt, h, :], sc[0:qn, :], AF.Exp,
                            scale=inv_sqrt_d,
                            accum_out=den[0:qn, qt, h:h + 1])
                        # interleave aux PE work between score tiles
                        if qt == 0:
                            if h > 0:
                                head_rec(h - 1)
                            drain(v_gen, vplan[h])
                        elif qt == 1 and h > 0:
                            head_g(h - 1)
                        elif qt == 3 and h >= 2:
                            head_pooled(h - 2)
                        else:
                            drain(aux, auxplan.get((h, qt), 0))
                head_rec(nh - 1)
                head_g(nh - 1)
                drain_all(v_gen)
                drain_all(aux)

                pooled_left = [nh - 2, nh - 1] if nh >= 2 else [0]
                tail_gen = branch_tail(vb_t, oww1, pooled_left)
                if mi + 1 == len(MHAS):
                    drain(tail_gen, 1)
                    warm()
                    drain(tail_gen, 1)
                    warm()
                    drain_all(tail_gen)
                    tail_gen = None

                # rotate per-branch state
                if mi + 1 < len(MHAS):
                    w_cur = w_nxt
                    qb_t, vb_t = qb_n, vb_n
                    mn, _ = MHAS[mi + 1]
                    oww1 = wres.tile([P, NT, H], BF16, tag="w_oww1",
                                     name=f"oww1_{mn}")
                    gated_dma(oww1, dram[f"{mn}_oww1"][:])

            # ---------------- MLP tail (all column-form) ------------------
            h1rc = small.tile([P, NT], BF16, tag="h1rc")
            nc.vector.tensor_relu(h1rc[:], h1acc[:])
            warm()
            tpg = pgp.tile([P, 4], F32, tag="g", name="tailpg")
            for oc in range(4):
                nc.tensor.matmul(tpg[:, oc:oc + 1],
                                 lhsT=b2r4[0:1, oc, :], rhs=oneb[:],
                                 start=True, stop=False)
                for kt in range(NT):
                    nc.tensor.matmul(
                        tpg[:, oc:oc + 1],
                        lhsT=f2r[:, kt, oc * P:(oc + 1) * P],
                        rhs=h1rc[:, kt:kt + 1],
                        start=False, stop=(kt == NT - 1))
            h2rc = small.tile([P, 4], BF16, tag="h2rc")
            nc.vector.tensor_relu(h2rc[:], tpg[:, 0:4])
            warm()

            plg = psc.tile([1, 2], F32, tag="sc", name="lg")
            for kt in range(4):
                nc.tensor.matmul(plg[:], lhsT=h2rc[:, kt:kt + 1],
                                 rhs=clsr[:, kt], start=(kt == 0),
                                 stop=(kt == 3))
            lg = small.tile([1, 2], F32, tag="lgsb")
            nc.vector.tensor_add(out=lg[:], in0=plg[:], in1=cbrow[:])
            nc.sync.dma_start(out[:], lg[:])

    _split_multi_waits(nc)
    return nc


def _split_multi_waits(nc, max_on_inst=1, max_on_evsem=2):
    """This walrus build caps sync waits per instruction at 1 (2 for
    EventSemaphore); Tile attaches one wait per dependent proc. Spill excess
    waits onto pure-wait EventSemaphores inserted before, on the same engine -
    the engine blocks on each condition in sequence, so semantics match."""
    for f in nc.m.functions:
        for bb in f.blocks:
            insts = list(bb.instructions)
            new = []
            changed = False
            for ins in insts:
                si = ins.sync_info
                if si is not None:
                    waits = list(si.on_wait)
                    cap = (max_on_evsem
                           if isinstance(ins, mybir.InstEventSemaphore)
                           else max_on_inst)
                    if len(waits) > cap:
                        spill = waits[:-cap]
                        keep = waits[-cap:]
                        k = 0
                        while spill:
                            chunk = spill[:max_on_evsem]
                            spill = spill[max_on_evsem:]
                            new.append(mybir.InstEventSemaphore(
                                name=f"{ins.name}-wspill{k}",
                                engine=ins.engine, ins=[], outs=[],
                                sync_info=mybir.SyncInfo(on_wait=chunk,
                                                         on_update=[])))
                            k += 1
                        ins.sync_info = mybir.SyncInfo(
                            on_wait=keep, on_update=list(si.on_update))
                        changed = True
                new.append(ins)
            if changed:
                bb.instructions = new


def _get_nc(Sp, Sq):
    if (Sp, Sq) not in _CACHE:
        _CACHE[(Sp, Sq)] = _build_nc(Sp, Sq)
    return _CACHE[(Sp, Sq)]


def _q8(a, scale):
    return np.clip(a.astype(np.float32) * scale, -240.0, 240.0).astype(E4)


def _h3(a):
    """[K, N] -> [P, K//P, N] partition-inner, contiguous."""
    K, N = a.shape
    return np.ascontiguousarray(a.reshape(K // P, P, N).transpose(1, 0, 2))


def _prep_in_maps(inputs, Sp, Sq):
    f32 = np.float32
    QT = -(-Sq // P)
    mask = inputs["attention_mask"].astype(np.int64)     # [B, S]

    w1 = inputs["fus1_w"].astype(f32)                    # [3H, H]
    shared = {
        "b2r4": np.ascontiguousarray(
            inputs["fus2_b"].astype(BF).reshape(1, 4, P)),
        "cbrow": inputs["cls_b"].astype(f32).reshape(1, 2),
        "fus2_w": _h3(inputs["fus2_w"].astype(BF)),
        "cls_w": _h3(inputs["cls_w"].astype(BF)),
    }

    b1 = inputs["fus1_b"].astype(f32).copy()
    for mi, (m, _) in enumerate(MHAS):
        w1m = w1[mi * H:(mi + 1) * H]                    # [H, H]
        for wn in ("qw", "kw", "vw"):
            shared[f"{m}_{wn}"] = _h3(_q8(inputs[f"{m}_{wn}"], SW))
        oww1 = inputs[f"{m}_ow"].astype(f32) @ w1m
        shared[f"{m}_oww1"] = _h3(oww1.astype(BF))
        b1 += inputs[f"{m}_ob"].astype(f32) @ w1m
        shared[f"{m}_qb"] = np.ascontiguousarray(
            inputs[f"{m}_qb"].astype(f32).reshape(NT, P).T)
        shared[f"{m}_vb"] = np.ascontiguousarray(
            inputs[f"{m}_vb"].astype(f32).reshape(NT, P).T)
    shared["b1col"] = np.ascontiguousarray(b1.reshape(NT, P).T)

    in_maps = []
    for c in range(NCORES):
        im = dict(shared)
        idx = np.nonzero(mask[c])[0]
        nv = len(idx)
        xp = np.zeros((Sp, H), f32)
        xp[:nv] = inputs["hidden_states"][c][idx]
        im["xT"] = _h3(_q8(xp.T, SX))
        pwv = np.zeros(QT * P, f32)
        pwv[:nv] = SWP / nv
        im["pw"] = np.ascontiguousarray(
            pwv.reshape(QT, P).T.astype(f32))
        im["npadneg"] = np.full((P, 1), -(Sp - nv), f32)
        in_maps.append(im)
    return in_maps


def kernel(**inputs) -> np.ndarray:
    mask = inputs["attention_mask"]
    maxc = int(mask.astype(np.int64).sum(axis=1).max())
    Sp = min(S, max(P, -(-maxc // P) * P))
    Sq = min(Sp, max(64, -(-maxc // 64) * 64))
    nc = _get_nc(Sp, Sq)
    in_maps = _prep_in_maps(inputs, Sp, Sq)
    res = run_bass_kernel_spmd(nc, in_maps, core_ids=list(range(NCORES)))
    return np.concatenate(
        [res.results[c]["out"] for c in range(NCORES)], axis=0
    ).astype(np.float32)
